# revision 80
# baseline (speedup 1.0000x reference)
"""Trainium2 Bass kernel for LongContextAttention (B=1, S=2048, H=16, D=128).

Strategy: permutations are pure data movement -> host-side numpy gathers.
Attention is head-parallel: 16 heads / 8 cores = 2 heads per core.

Per core, software-pipelined over 128 (h, q-chunk, k-tile) steps. The wall
clock decomposes as startup (~12us: framework preamble + first input DMA) +
a saturated ScalarE exp stream (~61us) + drain tail (~7us: last PV + fold +
store receipts), so the design keeps the exp stream gapless, offloads 1/16
of the exp work to the under-used DVE, and squeezes the two ends:
  - QK^T transposed: scoresT[k,q] = sum_d KT[d,k] QT[d,q]; stationary = KT
    tile (128x128), moving = QT chunk (128x512). PSUM banks rotate globally
    (seq%6, 6 banks) so the PE always runs ~6 k-tiles ahead of the exp.
  - exp on ScalarE for 15 of 16 k-tiles per chunk, in groups [3,3,3,3,2,1]
    so every group reads a contiguous, non-wrapping PSUM span for both bank
    phases (15*hq % 6 cycles 0,3); larger spans would stall at group
    boundaries (only 6-n banks of lookahead). One qk_sem wait per group plus
    one cp_sem pt-reuse wait per chunk keeps the scalar queue lean.
  - k-tile 14 of each chunk is exp'd on the DVE instead, via the Schraudolph
    fast-exp (int16(score*A+B) bit-cast as bf16, ~1.8% rms -> ~0.45% output
    error vs the 2e-2 budget). Its QK lands in the idle psum_cx spare bank
    (the one PV ping-pongs away from), so the slower DVE op never holds up
    the 6-bank rotation; dexp_sem guards the spare bank against the next
    chunk's PV and gates PV of tile 14 itself.
  - PE warm-up: 5 dummy matmuls at block entry push the HAM clock gate
    toward 2.4 GHz before the first real QK, and finish before its data
    lands so they add no latency.
  - PV with V as STATIONARY: psum_ctxT[d, q-chunk] accumulates over the 16
    k-tiles, moving = pt[:, kt, qchunk] (512 cols), 2 PSUM banks ping-pong.
  - denominator: 3-phase binary slab fold on DVE per chunk (8 tensor_adds,
    mostly FD=1024) phased inside that chunk's own exp window; the last
    chunk pre-folds planes 0:13 so only 2 FD=512 adds trail the final exp.
    s1[k, q] bf16 partials; the final 128-partition sum happens on the host.
  - ctxT PSUM -> SBUF copy on DVE converts to bf16; DMA out bf16 (halves the
    tail store). Host divides by the denominator and transposes (device
    output is ctx^T [h, d, q]).
  - input DMA: SDMA round-robins in-flight transfers at packet granularity,
    so the urgent slices are small/first (kq0 slab = qt chunk0 ++ kt tiles
    0:12, one contiguous DRAM source for large per-partition descriptors)
    and bulky loads are gated on pipeline progress (va0 split 4/12 tiles
    ungated, head-1 loads behind exp_sem >= 10) to keep them off the
    critical path without starving the in-order tensor engine's PV waits.
"""

import numpy as np
import ml_dtypes

B, S, H, D = 1, 2048, 16, 128
BLOCK = 64
NCORES = 8
HPC = H // NCORES          # heads per core = 2
NT = S // 128              # 16 k-tiles
NQC = 4                    # q-chunks of 512
QC = 512
SCALE = 1.0 / float(np.sqrt(D))
NSTEP = HPC * NQC * NT     # 128 (h, qc, kt) steps
NHQ = HPC * NQC            # 8 (h, qc) chunks
NQKBUF = 6                 # PSUM banks rotating for QK output

# Per chunk, k-tiles 14 AND 15 are offloaded to the DVE (fast-exp via the
# Schraudolph int16-bitcast trick); their QK outputs cycle through the idle
# psum_cx spare bank so the 6-bank rotation never waits on the slower DVE.
# ScalarE handles tiles 0..13 in wrap-free groups chosen per bank phase
# (14*hq % 6 cycles 0,2,4).
DVE_KT = 14                # first offloaded k-tile (14 and 15)
SPC = NT - 2               # scalar tiles per chunk = 14
PATTERNS = {0: [3, 3, 3, 3, 2], 2: [3, 1, 3, 3, 3, 1], 4: [2, 3, 3, 3, 3]}


def _pattern(hq):
    return PATTERNS[(SPC * hq) % NQKBUF]


# GRPS entries: (seq0, n, hq, kt0, ord) where seq0 is the chunk-global
# scalar sequence position (bank = seq % 6), kt0 the group's first k-tile
# (scalar tiles are 0..13 so kt == scalar index), ord the group's ordinal
# within its chunk.
GRPS = []
for _hq in range(NHQ):
    _s = 0
    for _o, _n in enumerate(_pattern(_hq)):
        _seq0 = SPC * _hq + _s
        assert (_seq0 % NQKBUF) + _n <= NQKBUF
        GRPS.append((_seq0, _n, _hq, _s, _o))
        _s += _n
    assert _s == SPC
GRPS_END = [0] * NHQ       # number of groups up to and including hq
for _gi, _g in enumerate(GRPS):
    GRPS_END[_g[2]] = max(GRPS_END[_g[2]], _gi + 1)

# Schraudolph fast-exp constants: i16 = round(score*A_FEXP + B_FEXP)
# bit-cast as bf16 equals exp(score*SCALE) with ~1.8% rms relative error
# (1/16 of all tiles -> ~0.45% output contribution; budget is 2e-2).
A_FEXP = float(SCALE * 128.0 / np.log(2.0))
B_FEXP = 127.0 * 128.0 - 7.375

# per-chunk fold phases: group count (within the chunk) after which >= 8 /
# >= 12 of the 16 k-planes are available.
FOLD_P1 = [0] * NHQ  # exp_sem value enabling planes 0:8
FOLD_P2 = [0] * NHQ  # exp_sem value enabling planes 0:12
FOLD_P2B = [0] * NHQ  # exp_sem value enabling planes 0:14
for _hq in range(NHQ):
    _cum = 0
    _base = GRPS_END[_hq - 1] if _hq > 0 else 0
    for _i, _n in enumerate(_pattern(_hq)):
        _cum += _n
        if FOLD_P1[_hq] == 0 and _cum >= 8:
            FOLD_P1[_hq] = _base + _i + 1
        if FOLD_P2[_hq] == 0 and _cum >= 12:
            FOLD_P2[_hq] = _base + _i + 1
        if FOLD_P2B[_hq] == 0 and _cum >= 14:
            FOLD_P2B[_hq] = _base + _i + 1

_CACHE = {}


def _build_bass():
    import concourse.bass as bass
    import concourse.mybir as mybir
    from contextlib import ExitStack

    f32 = mybir.dt.float32
    bf16 = mybir.dt.bfloat16

    nc = bass.Bass()
    kt_in = nc.declare_dram_parameter("kt_in", [HPC, D, S], bf16, isOutput=False)
    qt_in = nc.declare_dram_parameter("qt_in", [HPC, D, S], bf16, isOutput=False)
    va_in = nc.declare_dram_parameter("va_in", [HPC, 128, NT, D], bf16, isOutput=False)
    kq0_in = nc.declare_dram_parameter("kq0_in", [128, 2048], bf16, isOutput=False)
    ctx_out = nc.declare_dram_parameter("ctx_out", [HPC, D, S], bf16, isOutput=True)
    s_out = nc.declare_dram_parameter("s_out", [HPC, NQC, 128, QC], bf16, isOutput=True)

    ctx = ExitStack()
    with ctx:
        kt_sb = ctx.enter_context(nc.sbuf_tensor("kt_sb", [128, HPC, S], bf16))
        qt_sb = ctx.enter_context(nc.sbuf_tensor("qt_sb", [128, HPC, S], bf16))
        # prologue staging: qt0[:, 0:512] ++ kt0[:, 0:1536] (one DRAM slab ->
        # large per-partition descriptors -> fastest first-byte path)
        kq0_sb = ctx.enter_context(nc.sbuf_tensor("kq0_sb", [128, 2048], bf16))
        va_sb = ctx.enter_context(nc.sbuf_tensor("va_sb", [128, HPC, NT, D], bf16))
        pt_sb = [
            ctx.enter_context(nc.sbuf_tensor(f"pt_sb{i}", [128, NT, QC], bf16))
            for i in range(3)
        ]
        s2_sb = ctx.enter_context(nc.sbuf_tensor("s2_sb", [128, 8, QC], bf16))
        u_sb = ctx.enter_context(nc.sbuf_tensor("u_sb", [128, 4, QC], bf16))
        s1_sb = ctx.enter_context(nc.sbuf_tensor("s1_sb", [128, 2, QC], bf16))
        out_sb = ctx.enter_context(nc.sbuf_tensor("out_sb", [128, 2, QC], bf16))
        warm_sb = ctx.enter_context(nc.sbuf_tensor("warm_sb", [128, 2], f32))

        psum_qk = ctx.enter_context(nc.psum_tensor("psum_qk", [128, NQKBUF * QC], f32))
        psum_cx = ctx.enter_context(nc.psum_tensor("psum_cx", [128, 2 * QC], f32))

        load_sems = [ctx.enter_context(nc.semaphore(f"load_sem{i}")) for i in range(10)]
        qk_sem = ctx.enter_context(nc.semaphore("qk_sem"))
        exp_sem = ctx.enter_context(nc.semaphore("exp_sem"))
        sqk_sem = ctx.enter_context(nc.semaphore("sqk_sem"))
        dexp_sem = ctx.enter_context(nc.semaphore("dexp_sem"))
        pv_sem = ctx.enter_context(nc.semaphore("pv_sem"))
        tree_sem = ctx.enter_context(nc.semaphore("tree_sem"))
        cp_sem = ctx.enter_context(nc.semaphore("cp_sem"))
        st_sems = [ctx.enter_context(nc.semaphore(f"st_sem{i}")) for i in range(2)]
        s1st_sems = [ctx.enter_context(nc.semaphore(f"s1st_sem{i}")) for i in range(2)]

        block = ctx.enter_context(nc.Block(no_gpsimd_drain=True))

        @block.sync
        def _(sync):
            # Priority-phased input loads on the HWDGE (sync) path. The SDMA
            # engines round-robin between all queued transfers at packet
            # granularity, so lower-priority loads are gated behind qk_sem
            # progress to give the critical slices exclusive bandwidth.
            # Most urgent first: the staging slab slice QK(0..2) needs, then
            # the rest of the slab (tiles 3:12). Packet-level round-robin
            # favors small in-flight transfers, and the bulky va0/head-1
            # loads are additionally gated on pipeline progress.
            sync.dma_start(out=kq0_sb[:, 0:896], in_=kq0_in[:, 0:896]).then_inc(
                load_sems[0], 16
            )
            sync.dma_start(out=kq0_sb[:, 896:2048], in_=kq0_in[:, 896:2048]).then_inc(
                load_sems[7], 16
            )
            sync.dma_start(out=kt_sb[:, 0, 1536:S], in_=kt_in[0][:, 1536:S]).then_inc(
                load_sems[1], 16
            )
            sync.dma_start(out=qt_sb[:, 0, QC:S], in_=qt_in[0][:, QC:S]).then_inc(
                load_sems[3], 16
            )
            sync.dma_start(out=va_sb[:, 0, 0:4, :], in_=va_in[0][:, 0:4, :]).then_inc(
                load_sems[2], 16
            )
            sync.dma_start(out=va_sb[:, 0, 4:NT, :], in_=va_in[0][:, 4:NT, :]).then_inc(
                load_sems[8], 16
            )
            sync.wait_ge(exp_sem, 10)      # head-1 loads have ~30us of slack
            sync.dma_start(out=kt_sb[:, 1, :], in_=kt_in[1]).then_inc(load_sems[4], 16)
            sync.dma_start(out=qt_sb[:, 1, :], in_=qt_in[1]).then_inc(load_sems[6], 16)
            sync.dma_start(out=va_sb[:, 1, :, :], in_=va_in[1]).then_inc(load_sems[5], 16)
            # Output ctx DMAs from here (sync is idle after the loads);
            # the s1 DMAs stay on gpsimd so the triggers run in parallel.
            for hq in range(NHQ):
                h, qc = divmod(hq, NQC)
                sync.wait_ge(cp_sem, hq + 1)
                sync.dma_start(
                    out=ctx_out[h][:, qc * QC : (qc + 1) * QC],
                    in_=out_sb[:, hq % 2, :],
                ).then_inc(st_sems[hq % 2], 16)
            sync.wait_ge(st_sems[0], 16 * (NHQ // 2))
            sync.wait_ge(st_sems[1], 16 * (NHQ // 2))

        @block.tensor
        def _(tensor):
            def qk_operands(hq, kt):
                h, qc = divmod(hq, NQC)
                if h == 0 and kt < 12:
                    lhs = kq0_sb[:, 512 + kt * 128 : 512 + (kt + 1) * 128]
                else:
                    lhs = kt_sb[:, h, kt * 128 : (kt + 1) * 128]
                if h == 0 and qc == 0:
                    rhs = kq0_sb[:, 0:512]
                else:
                    rhs = qt_sb[:, h, qc * QC : (qc + 1) * QC]
                return lhs, rhs

            def emit_qk(seq):
                hq, kt = divmod(seq, SPC)
                if seq == 0:
                    tensor.wait_ge(load_sems[0], 16)   # kq0 first slice (qt c0 + kt 0:3)
                if seq == 3:
                    tensor.wait_ge(load_sems[7], 16)   # kq0 rest (kt 3:12)
                if seq == 12:
                    tensor.wait_ge(load_sems[1], 16)   # kt0 tiles 12:16
                if seq == SPC:
                    tensor.wait_ge(load_sems[3], 16)   # qt0 rest
                if seq == NQC * SPC:
                    tensor.wait_ge(load_sems[4], 16)   # kt1
                    tensor.wait_ge(load_sems[6], 16)   # qt1
                lhs, rhs = qk_operands(hq, kt)
                b = seq % NQKBUF
                tensor.matmul(
                    psum_qk[:, b * QC : (b + 1) * QC],
                    lhs,
                    rhs,
                    start=True,
                    stop=True,
                ).then_inc(qk_sem, 1)

            def emit_spare_qk(hq, kt):
                # an offloaded tile's QK goes to the idle psum_cx spare bank
                if kt == DVE_KT:
                    if hq >= 1:
                        tensor.wait_ge(cp_sem, hq)       # spare bank drained
                else:
                    tensor.wait_ge(dexp_sem, 2 * hq + 1)  # tile-14 TS read out
                lhs, rhs = qk_operands(hq, kt)
                sp = (hq + 1) % 2
                tensor.matmul(
                    psum_cx[:, sp * QC : (sp + 1) * QC],
                    lhs,
                    rhs,
                    start=True,
                    stop=True,
                ).then_inc(sqk_sem, 1)

            def emit_pv(hq2, kt2):
                h2 = hq2 // NQC
                cb = hq2 % 2
                pb2 = hq2 % 3
                if kt2 == 0:
                    tensor.wait_ge(load_sems[2] if h2 == 0 else load_sems[5], 16)
                    if hq2 >= 2:
                        tensor.wait_ge(cp_sem, hq2 - 1)  # psum_cx buf drained
                    if hq2 >= 1:
                        tensor.wait_ge(dexp_sem, 2 * hq2)  # spare bank read out
                if kt2 == 4 and hq2 == 0:
                    tensor.wait_ge(load_sems[8], 16)     # va0 tiles 4:16
                if kt2 >= DVE_KT:
                    tensor.wait_ge(dexp_sem, 2 * hq2 + (kt2 - DVE_KT) + 1)
                mm = tensor.matmul(
                    psum_cx[:, cb * QC : (cb + 1) * QC],
                    va_sb[:, h2, kt2, :],
                    pt_sb[pb2][:, kt2, :],
                    start=(kt2 == 0),
                    stop=(kt2 == NT - 1),
                )
                if kt2 == NT - 1:
                    mm.then_inc(pv_sem, 1)

            # Warm the PE HAM clock gate (1.2 -> 2.4 GHz needs ~3.4us of
            # sustained activity): dummy matmuls on resident SBUF garbage
            # into psum bank 0, overwritten later by QK g=0 (start=True).
            for _ in range(5):
                tensor.matmul(
                    psum_qk[:, 0:QC],
                    kq0_sb[:, 0:128],
                    kq0_sb[:, 0:QC],
                    start=True,
                    stop=True,
                )
            for seq in range(NQKBUF):
                emit_qk(seq)
            for gi, (s0, n, hq, kt0, o) in enumerate(GRPS):
                tensor.wait_ge(exp_sem, gi + 1)
                for seq in range(s0 + NQKBUF, min(s0 + NQKBUF + n, NHQ * SPC)):
                    emit_qk(seq)
                if o == 2:
                    emit_spare_qk(hq, DVE_KT)
                if o == 3:
                    emit_spare_qk(hq, DVE_KT + 1)
                for j in range(n):
                    emit_pv(hq, kt0 + j)
                if o == len(_pattern(hq)) - 1:
                    emit_pv(hq, DVE_KT)
                    emit_pv(hq, DVE_KT + 1)

        @block.scalar
        def _(scalar):
            import concourse.mybir as mybir_

            # Warm the Exp activation table immediately: the input is a
            # preamble-initialized const AP, so no waits and no memset chain.
            scalar.activation(
                out=warm_sb[:, 1:2],
                in_=nc.const_aps.tensor(0.0, (128, 1), mybir_.dt.float32),
                func=mybir_.ActivationFunctionType.Exp,
                scale=1.0,
            )
            for gi, (s0, n, hq, kt0, o) in enumerate(GRPS):
                if kt0 == 0 and hq >= 3:
                    # pt buf reused from (hq-3): cp_sem (the ctx copy) implies
                    # both PV and the denominator fold are done
                    scalar.wait_ge(cp_sem, hq - 2)
                scalar.wait_ge(qk_sem, s0 + n)
                b0 = s0 % NQKBUF
                scalar.activation(
                    out=pt_sb[hq % 3][:, kt0 : kt0 + n, :],
                    in_=psum_qk[:, b0 * QC : (b0 + n) * QC],
                    func=mybir_.ActivationFunctionType.Exp,
                    scale=SCALE,
                ).then_inc(exp_sem, 1)
            # The last chunk's ctxT copy runs here (scalar is idle after its
            # final exp; Copy shares the exp table set, so no table reload)
            # in parallel with the final denominator fold on the DVE.
            scalar.wait_ge(pv_sem, NHQ)
            scalar.wait_ge(st_sems[(NHQ - 1) % 2], 16 * ((NHQ - 1) // 2))
            scalar.copy(
                out=out_sb[:, (NHQ - 1) % 2, :],
                in_=psum_cx[:, ((NHQ - 1) % 2) * QC : ((NHQ - 1) % 2 + 1) * QC],
            ).then_inc(cp_sem, 1)

        @block.vector
        def _(vector):
            # Denominator: per chunk, a binary slab fold of the 16 exp planes
            # (mostly FD=1024 tensor_adds), phased in 3 steps so the bulk
            # overlaps the chunk's own exp stream:
            #   P1 (>=8 planes):  a = p0:2+p2:4 -> s2[0:2]; b = p4:6+p6:8 -> s2[2:4]
            #   P2 (>=12 planes): c = p8:10+p10:12 -> s2[4:6]; u0 = a+b -> u[0:2]
            #   P3 (all 16):      d = p12:14+p14:16 -> s2[6:8]; u1 = c+d -> u[2:4];
            #                     w = u0+u1 -> s2[0:2]; s1 = w0+w1
            # then the ctxT PSUM -> SBUF (bf16) staging copy.
            for hq in range(NHQ):
                pb = hq % 3
                sb1 = hq % 2
                cb = hq % 2
                last = hq == NHQ - 1
                pt = pt_sb[pb]
                # this chunk's offloaded k-tiles 14/15: fast-exp on the DVE
                # from the psum_cx spare bank (outside the 6-bank rotation)
                sp = (hq + 1) % 2
                for dk in (DVE_KT, DVE_KT + 1):
                    vector.wait_ge(sqk_sem, 2 * hq + (dk - DVE_KT) + 1)
                    vector.tensor_scalar(
                        out=pt[:, dk : dk + 1, :].bitcast(mybir.dt.int16),
                        in0=psum_cx[:, sp * QC : (sp + 1) * QC],
                        scalar1=A_FEXP,
                        scalar2=B_FEXP,
                        op0=mybir.AluOpType.mult,
                        op1=mybir.AluOpType.add,
                    ).then_inc(dexp_sem, 1)
                with nc.allow_low_precision("bf16 partials; host fp32 final sum"):
                    vector.wait_ge(exp_sem, FOLD_P1[hq])
                    vector.tensor_add(
                        out=s2_sb[:, 0:2, :], in0=pt[:, 0:2, :], in1=pt[:, 2:4, :]
                    )
                    vector.tensor_add(
                        out=s2_sb[:, 2:4, :], in0=pt[:, 4:6, :], in1=pt[:, 6:8, :]
                    )
                    vector.wait_ge(exp_sem, FOLD_P2[hq])
                    vector.tensor_add(
                        out=s2_sb[:, 4:6, :], in0=pt[:, 8:10, :], in1=pt[:, 10:12, :]
                    )
                    vector.tensor_add(
                        out=u_sb[:, 0:2, :], in0=s2_sb[:, 0:2, :], in1=s2_sb[:, 2:4, :]
                    )
                    if last:
                        # pre-compute m = p0..11 and t = p14+p15 (DVE planes,
                        # ready mid-window) so only 3 small FD=512 adds remain
                        # after the final exp group: d1 = p12+p13; q = m+d1;
                        # s1 = q+t
                        vector.tensor_add(
                            out=u_sb[:, 3, :], in0=s2_sb[:, 4, :], in1=s2_sb[:, 5, :]
                        )
                        vector.tensor_add(
                            out=s2_sb[:, 6, :], in0=u_sb[:, 0, :], in1=u_sb[:, 1, :]
                        )
                        vector.tensor_add(
                            out=s2_sb[:, 5, :], in0=u_sb[:, 3, :], in1=s2_sb[:, 6, :]
                        )
                        vector.tensor_add(
                            out=s2_sb[:, 7, :], in0=pt[:, 14, :], in1=pt[:, 15, :]
                        )
                    # final fold phase first: it only needs the chunk's last
                    # exp group (~3us before PV+copy), so the s1 store never
                    # backs up behind the pv-gated ctx copy.
                    vector.wait_ge(exp_sem, GRPS_END[hq])
                    if hq >= 2:
                        vector.wait_ge(s1st_sems[hq % 2], 16 * (hq // 2))
                    if last:
                        vector.tensor_add(
                            out=u_sb[:, 2, :], in0=pt[:, 12, :], in1=pt[:, 13, :]
                        )
                        vector.tensor_add(
                            out=s2_sb[:, 4, :], in0=u_sb[:, 2, :], in1=s2_sb[:, 5, :]
                        )
                        vector.tensor_add(
                            out=s1_sb[:, sb1, :], in0=s2_sb[:, 4, :], in1=s2_sb[:, 7, :]
                        ).then_inc(tree_sem, 1)
                    else:
                        vector.tensor_add(
                            out=s2_sb[:, 6:8, :], in0=pt[:, 12:14, :], in1=pt[:, 14:16, :]
                        )
                        vector.tensor_add(
                            out=u_sb[:, 2:4, :], in0=s2_sb[:, 4:6, :], in1=s2_sb[:, 6:8, :]
                        )
                        vector.tensor_add(
                            out=s2_sb[:, 0:2, :], in0=u_sb[:, 0:2, :], in1=u_sb[:, 2:4, :]
                        )
                        vector.tensor_add(
                            out=s1_sb[:, sb1, :], in0=s2_sb[:, 0, :], in1=s2_sb[:, 1, :]
                        ).then_inc(tree_sem, 1)
                    # ctxT PSUM -> SBUF bf16 staging copy (the last chunk's
                    # copy is done by the scalar engine instead)
                    if not last:
                        vector.wait_ge(pv_sem, hq + 1)
                        if hq >= 2:
                            vector.wait_ge(st_sems[hq % 2], 16 * (hq // 2))
                        vector.tensor_scalar_add(
                            out=out_sb[:, cb, :],
                            in0=psum_cx[:, cb * QC : (cb + 1) * QC],
                            scalar1=0.0,
                        ).then_inc(cp_sem, 1)

        @block.gpsimd
        def _(gpsimd):
            for hq in range(NHQ):
                h, qc = divmod(hq, NQC)
                gpsimd.wait_ge(tree_sem, hq + 1)
                gpsimd.dma_start(
                    out=s_out[h, qc], in_=s1_sb[:, hq % 2, :]
                ).then_inc(s1st_sems[hq % 2], 16)
            gpsimd.wait_ge(s1st_sems[0], 16 * (NHQ // 2))
            gpsimd.wait_ge(s1st_sems[1], 16 * (NHQ // 2))

    return nc


def _perm_blocks(x, idx):
    xb = x.reshape(B, S // BLOCK, BLOCK, H, D)
    return xb[:, idx].reshape(B, S, H, D)


def kernel(**inputs):
    from concourse.bass_utils import run_bass_kernel_spmd

    q = np.asarray(inputs["query"], dtype=np.float32)
    k = np.asarray(inputs["key"], dtype=np.float32)
    v = np.asarray(inputs["value"], dtype=np.float32)
    hp = np.asarray(inputs["head_perm_idx"]).astype(np.int64)
    hd = np.asarray(inputs["head_deperm_idx"]).astype(np.int64)
    rp = np.asarray(inputs["new_row_perm_idx"]).astype(np.int64)
    cp = np.asarray(inputs["new_col_perm_idx"]).astype(np.int64)
    rd = np.asarray(inputs["new_row_deperm_idx"]).astype(np.int64)

    qp = _perm_blocks(q[:, :, hp], rp)[0]  # [S, H, D]
    kp = _perm_blocks(k[:, :, hp], cp)[0]
    vp = _perm_blocks(v[:, :, hp], cp)[0]

    bf = ml_dtypes.bfloat16
    qt = np.ascontiguousarray(qp.transpose(1, 2, 0)).astype(bf)  # [H, D, S]
    kt = np.ascontiguousarray(kp.transpose(1, 2, 0)).astype(bf)  # [H, D, S]
    # va[h, kp, kt, d] = V[h, kt*128 + kp, d]
    va = np.ascontiguousarray(
        vp.transpose(1, 0, 2).reshape(H, NT, 128, D).transpose(0, 2, 1, 3)
    ).astype(bf)

    if "nc" not in _CACHE:
        _CACHE["nc"] = _build_bass()
    nc = _CACHE["nc"]

    core_ids = list(range(NCORES))
    in_maps = [
        {
            "kt_in": np.ascontiguousarray(kt[c * HPC : (c + 1) * HPC]),
            "qt_in": np.ascontiguousarray(qt[c * HPC : (c + 1) * HPC]),
            "va_in": np.ascontiguousarray(va[c * HPC : (c + 1) * HPC]),
            "kq0_in": np.ascontiguousarray(
                np.concatenate(
                    [qt[c * HPC][:, 0:QC], kt[c * HPC][:, 0:1536]], axis=1
                )
            ),
        }
        for c in core_ids
    ]
    res = run_bass_kernel_spmd(nc, in_maps, core_ids)
    _CACHE["last_result"] = res

    ctxT = np.concatenate(
        [np.asarray(res.results[c]["ctx_out"], dtype=np.float32) for c in core_ids],
        axis=0,
    )  # [H, D, S] fp32 (from bf16), unnormalized
    s1 = np.concatenate(
        [np.asarray(res.results[c]["s_out"], dtype=np.float32) for c in core_ids],
        axis=0,
    )  # [H, NQC, 128, QC]
    denom = s1.sum(axis=2).reshape(H, S)  # [H, S]
    ctxT = ctxT / denom[:, None, :]
    ctx = np.ascontiguousarray(ctxT.transpose(2, 0, 1))[None]  # [1, S, H, D]
    ctx = _perm_blocks(ctx, rd)
    out = ctx[:, :, hd]
    return np.ascontiguousarray(out, dtype=np.float32)


# revision 83
# speedup vs baseline: 1.0224x; 1.0224x over previous
"""Trainium2 Bass kernel for LongContextAttention (B=1, S=2048, H=16, D=128).

Strategy: permutations are pure data movement -> host-side numpy gathers.
Attention is head-parallel: 16 heads / 8 cores = 2 heads per core.

Per core, software-pipelined over 128 (h, q-chunk, k-tile) steps. The wall
clock decomposes as startup (~12us: framework preamble + first input DMA) +
a saturated ScalarE exp stream (~61us) + drain tail (~7us: last PV + fold +
store receipts), so the design keeps the exp stream gapless, offloads 1/16
of the exp work to the under-used DVE, and squeezes the two ends:
  - QK^T transposed: scoresT[k,q] = sum_d KT[d,k] QT[d,q]; stationary = KT
    tile (128x128), moving = QT chunk (128x512). PSUM banks rotate globally
    (seq%6, 6 banks) so the PE always runs ~6 k-tiles ahead of the exp.
  - exp on ScalarE for 15 of 16 k-tiles per chunk, in groups [3,3,3,3,2,1]
    so every group reads a contiguous, non-wrapping PSUM span for both bank
    phases (15*hq % 6 cycles 0,3); larger spans would stall at group
    boundaries (only 6-n banks of lookahead). One qk_sem wait per group plus
    one cp_sem pt-reuse wait per chunk keeps the scalar queue lean.
  - k-tile 14 of each chunk is exp'd on the DVE instead, via the Schraudolph
    fast-exp (int16(score*A+B) bit-cast as bf16, ~1.8% rms -> ~0.45% output
    error vs the 2e-2 budget). Its QK lands in the idle psum_cx spare bank
    (the one PV ping-pongs away from), so the slower DVE op never holds up
    the 6-bank rotation; dexp_sem guards the spare bank against the next
    chunk's PV and gates PV of tile 14 itself.
  - PE warm-up: 5 dummy matmuls at block entry push the HAM clock gate
    toward 2.4 GHz before the first real QK, and finish before its data
    lands so they add no latency.
  - PV with V as STATIONARY: psum_ctxT[d, q-chunk] accumulates over the 16
    k-tiles, moving = pt[:, kt, qchunk] (512 cols), 2 PSUM banks ping-pong.
  - denominator: 3-phase binary slab fold on DVE per chunk (8 tensor_adds,
    mostly FD=1024) phased inside that chunk's own exp window; the last
    chunk pre-folds planes 0:13 so only 2 FD=512 adds trail the final exp.
    s1[k, q] bf16 partials; the final 128-partition sum happens on the host.
  - ctxT PSUM -> SBUF copy on DVE converts to bf16; DMA out bf16 (halves the
    tail store). Host divides by the denominator and transposes (device
    output is ctx^T [h, d, q]).
  - input DMA: SDMA round-robins in-flight transfers at packet granularity,
    so the urgent slices are small/first (kq0 slab = qt chunk0 ++ kt tiles
    0:12, one contiguous DRAM source for large per-partition descriptors)
    and bulky loads are gated on pipeline progress (va0 split 4/12 tiles
    ungated, head-1 loads behind exp_sem >= 10) to keep them off the
    critical path without starving the in-order tensor engine's PV waits.
"""

import numpy as np
import ml_dtypes

B, S, H, D = 1, 2048, 16, 128
BLOCK = 64
NCORES = 8
HPC = H // NCORES          # heads per core = 2
NT = S // 128              # 16 k-tiles
NQC = 4                    # q-chunks of 512
QC = 512
SCALE = 1.0 / float(np.sqrt(D))
NSTEP = HPC * NQC * NT     # 128 (h, qc, kt) steps
NHQ = HPC * NQC            # 8 (h, qc) chunks
NQKBUF = 6                 # PSUM banks rotating for QK output

# Per chunk, k-tiles 14 AND 15 are offloaded to the DVE (fast-exp via the
# Schraudolph int16-bitcast trick); their QK outputs cycle through the idle
# psum_cx spare bank so the 6-bank rotation never waits on the slower DVE.
# ScalarE handles tiles 0..13 in wrap-free groups chosen per bank phase
# (14*hq % 6 cycles 0,2,4).
DVE_KT = 14                # first offloaded k-tile (14 and 15)
SPC = NT - 2               # scalar tiles per chunk = 14
PATTERNS = {0: [3, 3, 3, 3, 2], 2: [3, 1, 3, 3, 3, 1], 4: [2, 3, 3, 3, 3]}


def _pattern(hq):
    return PATTERNS[(SPC * hq) % NQKBUF]


# GRPS entries: (seq0, n, hq, kt0, ord) where seq0 is the chunk-global
# scalar sequence position (bank = seq % 6), kt0 the group's first k-tile
# (scalar tiles are 0..13 so kt == scalar index), ord the group's ordinal
# within its chunk.
GRPS = []
for _hq in range(NHQ):
    _s = 0
    for _o, _n in enumerate(_pattern(_hq)):
        _seq0 = SPC * _hq + _s
        assert (_seq0 % NQKBUF) + _n <= NQKBUF
        GRPS.append((_seq0, _n, _hq, _s, _o))
        _s += _n
    assert _s == SPC
GRPS_END = [0] * NHQ       # number of groups up to and including hq
for _gi, _g in enumerate(GRPS):
    GRPS_END[_g[2]] = max(GRPS_END[_g[2]], _gi + 1)

# Schraudolph fast-exp constants: i16 = round(score*A_FEXP + B_FEXP)
# bit-cast as bf16 equals exp(score*SCALE) with ~1.8% rms relative error
# (1/16 of all tiles -> ~0.45% output contribution; budget is 2e-2).
A_FEXP = float(SCALE * 128.0 / np.log(2.0))
B_FEXP = 127.0 * 128.0 - 7.375

# per-chunk fold phases: group count (within the chunk) after which >= 8 /
# >= 12 of the 16 k-planes are available.
FOLD_P1 = [0] * NHQ  # exp_sem value enabling planes 0:8
FOLD_P2 = [0] * NHQ  # exp_sem value enabling planes 0:12
FOLD_P2B = [0] * NHQ  # exp_sem value enabling planes 0:14
for _hq in range(NHQ):
    _cum = 0
    _base = GRPS_END[_hq - 1] if _hq > 0 else 0
    for _i, _n in enumerate(_pattern(_hq)):
        _cum += _n
        if FOLD_P1[_hq] == 0 and _cum >= 8:
            FOLD_P1[_hq] = _base + _i + 1
        if FOLD_P2[_hq] == 0 and _cum >= 12:
            FOLD_P2[_hq] = _base + _i + 1
        if FOLD_P2B[_hq] == 0 and _cum >= 14:
            FOLD_P2B[_hq] = _base + _i + 1

_CACHE = {}


def _build_bass():
    import concourse.bass as bass
    import concourse.mybir as mybir
    from contextlib import ExitStack

    f32 = mybir.dt.float32
    bf16 = mybir.dt.bfloat16

    nc = bass.Bass()
    kt_in = nc.declare_dram_parameter("kt_in", [HPC, D, S], bf16, isOutput=False)
    qt_in = nc.declare_dram_parameter("qt_in", [HPC, D, S], bf16, isOutput=False)
    va_in = nc.declare_dram_parameter("va_in", [HPC, 128, NT, D], bf16, isOutput=False)
    kq0_in = nc.declare_dram_parameter("kq0_in", [128, 2048], bf16, isOutput=False)
    ctx_out = nc.declare_dram_parameter("ctx_out", [HPC, D, S], bf16, isOutput=True)
    s_out = nc.declare_dram_parameter("s_out", [HPC, NQC, 128, QC], bf16, isOutput=True)

    ctx = ExitStack()
    with ctx:
        kt_sb = ctx.enter_context(nc.sbuf_tensor("kt_sb", [128, HPC, S], bf16))
        qt_sb = ctx.enter_context(nc.sbuf_tensor("qt_sb", [128, HPC, S], bf16))
        # prologue staging: qt0[:, 0:512] ++ kt0[:, 0:1536] (one DRAM slab ->
        # large per-partition descriptors -> fastest first-byte path)
        kq0_sb = ctx.enter_context(nc.sbuf_tensor("kq0_sb", [128, 2048], bf16))
        va_sb = ctx.enter_context(nc.sbuf_tensor("va_sb", [128, HPC, NT, D], bf16))
        pt_sb = [
            ctx.enter_context(nc.sbuf_tensor(f"pt_sb{i}", [128, NT, QC], bf16))
            for i in range(3)
        ]
        s2_sb = ctx.enter_context(nc.sbuf_tensor("s2_sb", [128, 8, QC], bf16))
        u_sb = ctx.enter_context(nc.sbuf_tensor("u_sb", [128, 4, QC], bf16))
        s1_sb = ctx.enter_context(nc.sbuf_tensor("s1_sb", [128, 2, QC], bf16))
        out_sb = ctx.enter_context(nc.sbuf_tensor("out_sb", [128, 2, QC], bf16))
        warm_sb = ctx.enter_context(nc.sbuf_tensor("warm_sb", [128, 2], f32))

        psum_qk = ctx.enter_context(nc.psum_tensor("psum_qk", [128, NQKBUF * QC], f32))
        psum_cx = ctx.enter_context(nc.psum_tensor("psum_cx", [128, 2 * QC], f32))

        load_sems = [ctx.enter_context(nc.semaphore(f"load_sem{i}")) for i in range(10)]
        qk_sem = ctx.enter_context(nc.semaphore("qk_sem"))
        exp_sem = ctx.enter_context(nc.semaphore("exp_sem"))
        sqk_sem = ctx.enter_context(nc.semaphore("sqk_sem"))
        dexp_sem = ctx.enter_context(nc.semaphore("dexp_sem"))
        pv_sem = ctx.enter_context(nc.semaphore("pv_sem"))
        tree_sem = ctx.enter_context(nc.semaphore("tree_sem"))
        cp_sem = ctx.enter_context(nc.semaphore("cp_sem"))
        st_sems = [ctx.enter_context(nc.semaphore(f"st_sem{i}")) for i in range(2)]
        s1st_sems = [ctx.enter_context(nc.semaphore(f"s1st_sem{i}")) for i in range(2)]

        block = ctx.enter_context(nc.Block(no_gpsimd_drain=True))

        @block.sync
        def _(sync):
            # Priority-phased input loads on the HWDGE (sync) path. The SDMA
            # engines round-robin between all queued transfers at packet
            # granularity, so lower-priority loads are gated behind qk_sem
            # progress to give the critical slices exclusive bandwidth.
            # Most urgent first: the staging slab slice QK(0..2) needs, then
            # the rest of the slab (tiles 3:12). Packet-level round-robin
            # favors small in-flight transfers, and the bulky va0/head-1
            # loads are additionally gated on pipeline progress.
            sync.dma_start(out=kq0_sb[:, 0:896], in_=kq0_in[:, 0:896]).then_inc(
                load_sems[0], 16
            )
            sync.dma_start(out=kq0_sb[:, 896:2048], in_=kq0_in[:, 896:2048]).then_inc(
                load_sems[7], 16
            )
            sync.dma_start(out=kt_sb[:, 0, 1536:S], in_=kt_in[0][:, 1536:S]).then_inc(
                load_sems[1], 16
            )
            sync.dma_start(out=qt_sb[:, 0, QC:S], in_=qt_in[0][:, QC:S]).then_inc(
                load_sems[3], 16
            )
            sync.dma_start(out=va_sb[:, 0, 0:4, :], in_=va_in[0][:, 0:4, :]).then_inc(
                load_sems[2], 16
            )
            sync.dma_start(out=va_sb[:, 0, 4:NT, :], in_=va_in[0][:, 4:NT, :]).then_inc(
                load_sems[8], 16
            )
            sync.wait_ge(exp_sem, 10)      # head-1 loads have ~30us of slack
            sync.dma_start(out=kt_sb[:, 1, :], in_=kt_in[1]).then_inc(load_sems[4], 16)
            sync.dma_start(out=qt_sb[:, 1, :], in_=qt_in[1]).then_inc(load_sems[6], 16)
            sync.dma_start(out=va_sb[:, 1, :, :], in_=va_in[1]).then_inc(load_sems[5], 16)
            # Output ctx DMAs from here (sync is idle after the loads);
            # the s1 DMAs stay on gpsimd so the triggers run in parallel.
            for hq in range(NHQ):
                h, qc = divmod(hq, NQC)
                sync.wait_ge(cp_sem, hq + 1)
                sync.dma_start(
                    out=ctx_out[h][:, qc * QC : (qc + 1) * QC],
                    in_=out_sb[:, hq % 2, :],
                ).then_inc(st_sems[hq % 2], 16)
            sync.wait_ge(st_sems[0], 16 * (NHQ // 2))
            sync.wait_ge(st_sems[1], 16 * (NHQ // 2))

        @block.tensor
        def _(tensor):
            def qk_operands(hq, kt):
                h, qc = divmod(hq, NQC)
                if h == 0 and kt < 12:
                    lhs = kq0_sb[:, 512 + kt * 128 : 512 + (kt + 1) * 128]
                else:
                    lhs = kt_sb[:, h, kt * 128 : (kt + 1) * 128]
                if h == 0 and qc == 0:
                    rhs = kq0_sb[:, 0:512]
                else:
                    rhs = qt_sb[:, h, qc * QC : (qc + 1) * QC]
                return lhs, rhs

            def emit_qk(seq):
                hq, kt = divmod(seq, SPC)
                if seq == 0:
                    tensor.wait_ge(load_sems[0], 16)   # kq0 first slice (qt c0 + kt 0:3)
                if seq == 3:
                    tensor.wait_ge(load_sems[7], 16)   # kq0 rest (kt 3:12)
                if seq == 12:
                    tensor.wait_ge(load_sems[1], 16)   # kt0 tiles 12:16
                if seq == SPC:
                    tensor.wait_ge(load_sems[3], 16)   # qt0 rest
                if seq == NQC * SPC:
                    tensor.wait_ge(load_sems[4], 16)   # kt1
                    tensor.wait_ge(load_sems[6], 16)   # qt1
                lhs, rhs = qk_operands(hq, kt)
                b = seq % NQKBUF
                tensor.matmul(
                    psum_qk[:, b * QC : (b + 1) * QC],
                    lhs,
                    rhs,
                    start=True,
                    stop=True,
                ).then_inc(qk_sem, 1)

            def emit_spare_qk(hq, kt):
                # an offloaded tile's QK goes to the idle psum_cx spare bank
                if kt == DVE_KT:
                    if hq >= 1:
                        tensor.wait_ge(cp_sem, hq)       # spare bank drained
                else:
                    tensor.wait_ge(dexp_sem, 2 * hq + 1)  # tile-14 TS read out
                lhs, rhs = qk_operands(hq, kt)
                sp = (hq + 1) % 2
                tensor.matmul(
                    psum_cx[:, sp * QC : (sp + 1) * QC],
                    lhs,
                    rhs,
                    start=True,
                    stop=True,
                ).then_inc(sqk_sem, 1)

            def emit_pv(hq2, kt2):
                h2 = hq2 // NQC
                cb = hq2 % 2
                pb2 = hq2 % 3
                if kt2 == 0:
                    tensor.wait_ge(load_sems[2] if h2 == 0 else load_sems[5], 16)
                    if hq2 >= 2:
                        tensor.wait_ge(cp_sem, hq2 - 1)  # psum_cx buf drained
                    if hq2 >= 1:
                        tensor.wait_ge(dexp_sem, 2 * hq2)  # spare bank read out
                if kt2 == 4 and hq2 == 0:
                    tensor.wait_ge(load_sems[8], 16)     # va0 tiles 4:16
                if kt2 >= DVE_KT:
                    tensor.wait_ge(dexp_sem, 2 * hq2 + (kt2 - DVE_KT) + 1)
                mm = tensor.matmul(
                    psum_cx[:, cb * QC : (cb + 1) * QC],
                    va_sb[:, h2, kt2, :],
                    pt_sb[pb2][:, kt2, :],
                    start=(kt2 == 0),
                    stop=(kt2 == NT - 1),
                )
                if kt2 == NT - 1:
                    mm.then_inc(pv_sem, 1)

            # Warm the PE HAM clock gate (1.2 -> 2.4 GHz needs ~3.4us of
            # sustained activity): dummy matmuls on resident SBUF garbage
            # into psum bank 0, overwritten later by QK g=0 (start=True).
            for _ in range(5):
                tensor.matmul(
                    psum_qk[:, 0:QC],
                    kq0_sb[:, 0:128],
                    kq0_sb[:, 0:QC],
                    start=True,
                    stop=True,
                )
            for seq in range(NQKBUF):
                emit_qk(seq)
            for gi, (s0, n, hq, kt0, o) in enumerate(GRPS):
                tensor.wait_ge(exp_sem, gi + 1)
                for seq in range(s0 + NQKBUF, min(s0 + NQKBUF + n, NHQ * SPC)):
                    emit_qk(seq)
                if o == 2:
                    emit_spare_qk(hq, DVE_KT)
                if o == len(_pattern(hq)) - 1:
                    emit_spare_qk(hq, DVE_KT + 1)
                for j in range(n):
                    emit_pv(hq, kt0 + j)
                if o == len(_pattern(hq)) - 1:
                    emit_pv(hq, DVE_KT)
                    emit_pv(hq, DVE_KT + 1)

        @block.scalar
        def _(scalar):
            import concourse.mybir as mybir_

            # Warm the Exp activation table immediately: the input is a
            # preamble-initialized const AP, so no waits and no memset chain.
            scalar.activation(
                out=warm_sb[:, 1:2],
                in_=nc.const_aps.tensor(0.0, (128, 1), mybir_.dt.float32),
                func=mybir_.ActivationFunctionType.Exp,
                scale=1.0,
            )
            for gi, (s0, n, hq, kt0, o) in enumerate(GRPS):
                if kt0 == 0 and hq >= 3:
                    # pt buf reused from (hq-3): cp_sem (the ctx copy) implies
                    # both PV and the denominator fold are done
                    scalar.wait_ge(cp_sem, hq - 2)
                scalar.wait_ge(qk_sem, s0 + n)
                b0 = s0 % NQKBUF
                scalar.activation(
                    out=pt_sb[hq % 3][:, kt0 : kt0 + n, :],
                    in_=psum_qk[:, b0 * QC : (b0 + n) * QC],
                    func=mybir_.ActivationFunctionType.Exp,
                    scale=SCALE,
                ).then_inc(exp_sem, 1)
            # The last chunk's ctxT copy runs here (scalar is idle after its
            # final exp; Copy shares the exp table set, so no table reload)
            # in parallel with the final denominator fold on the DVE.
            scalar.wait_ge(pv_sem, NHQ)
            scalar.wait_ge(st_sems[(NHQ - 1) % 2], 16 * ((NHQ - 1) // 2))
            scalar.copy(
                out=out_sb[:, (NHQ - 1) % 2, :],
                in_=psum_cx[:, ((NHQ - 1) % 2) * QC : ((NHQ - 1) % 2 + 1) * QC],
            ).then_inc(cp_sem, 1)

        @block.vector
        def _(vector):
            # Denominator: per chunk, a binary slab fold of the 16 exp planes
            # (mostly FD=1024 tensor_adds), phased in 3 steps so the bulk
            # overlaps the chunk's own exp stream:
            #   P1 (>=8 planes):  a = p0:2+p2:4 -> s2[0:2]; b = p4:6+p6:8 -> s2[2:4]
            #   P2 (>=12 planes): c = p8:10+p10:12 -> s2[4:6]; u0 = a+b -> u[0:2]
            #   P3 (all 16):      d = p12:14+p14:16 -> s2[6:8]; u1 = c+d -> u[2:4];
            #                     w = u0+u1 -> s2[0:2]; s1 = w0+w1
            # then the ctxT PSUM -> SBUF (bf16) staging copy.
            for hq in range(NHQ):
                pb = hq % 3
                sb1 = hq % 2
                cb = hq % 2
                last = hq == NHQ - 1
                pt = pt_sb[pb]
                # this chunk's offloaded k-tiles 14/15: fast-exp on the DVE
                # from the psum_cx spare bank (outside the 6-bank rotation)
                sp = (hq + 1) % 2

                def dve_exp(dk):
                    vector.wait_ge(sqk_sem, 2 * hq + (dk - DVE_KT) + 1)
                    vector.tensor_scalar(
                        out=pt[:, dk : dk + 1, :].bitcast(mybir.dt.int16),
                        in0=psum_cx[:, sp * QC : (sp + 1) * QC],
                        scalar1=A_FEXP,
                        scalar2=B_FEXP,
                        op0=mybir.AluOpType.mult,
                        op1=mybir.AluOpType.add,
                    ).then_inc(dexp_sem, 1)

                dve_exp(DVE_KT)
                with nc.allow_low_precision("bf16 partials; host fp32 final sum"):
                    vector.wait_ge(exp_sem, FOLD_P1[hq])
                    vector.tensor_add(
                        out=s2_sb[:, 0:2, :], in0=pt[:, 0:2, :], in1=pt[:, 2:4, :]
                    )
                    vector.tensor_add(
                        out=s2_sb[:, 2:4, :], in0=pt[:, 4:6, :], in1=pt[:, 6:8, :]
                    )
                    vector.wait_ge(exp_sem, FOLD_P2[hq])
                    vector.tensor_add(
                        out=s2_sb[:, 4:6, :], in0=pt[:, 8:10, :], in1=pt[:, 10:12, :]
                    )
                    vector.tensor_add(
                        out=u_sb[:, 0:2, :], in0=s2_sb[:, 0:2, :], in1=s2_sb[:, 2:4, :]
                    )
                    if last:
                        # pre-compute m = p0..11 and t = p14+p15 (DVE planes,
                        # ready mid-window) so only 3 small FD=512 adds remain
                        # after the final exp group: d1 = p12+p13; q = m+d1;
                        # s1 = q+t
                        vector.tensor_add(
                            out=u_sb[:, 3, :], in0=s2_sb[:, 4, :], in1=s2_sb[:, 5, :]
                        )
                        vector.tensor_add(
                            out=s2_sb[:, 6, :], in0=u_sb[:, 0, :], in1=u_sb[:, 1, :]
                        )
                        vector.tensor_add(
                            out=s2_sb[:, 5, :], in0=u_sb[:, 3, :], in1=s2_sb[:, 6, :]
                        )
                    # tile-15 fast-exp here (its spare QK is emitted near the
                    # chunk's last group, after the tile-14 TS freed the bank)
                    dve_exp(DVE_KT + 1)
                    if last:
                        vector.tensor_add(
                            out=s2_sb[:, 7, :], in0=pt[:, 14, :], in1=pt[:, 15, :]
                        )
                    # final fold phase first: it only needs the chunk's last
                    # exp group (~3us before PV+copy), so the s1 store never
                    # backs up behind the pv-gated ctx copy.
                    vector.wait_ge(exp_sem, GRPS_END[hq])
                    if hq >= 2:
                        vector.wait_ge(s1st_sems[hq % 2], 16 * (hq // 2))
                    if last:
                        vector.tensor_add(
                            out=u_sb[:, 2, :], in0=pt[:, 12, :], in1=pt[:, 13, :]
                        )
                        vector.tensor_add(
                            out=s2_sb[:, 4, :], in0=u_sb[:, 2, :], in1=s2_sb[:, 5, :]
                        )
                        vector.tensor_add(
                            out=s1_sb[:, sb1, :], in0=s2_sb[:, 4, :], in1=s2_sb[:, 7, :]
                        ).then_inc(tree_sem, 1)
                    else:
                        vector.tensor_add(
                            out=s2_sb[:, 6:8, :], in0=pt[:, 12:14, :], in1=pt[:, 14:16, :]
                        )
                        vector.tensor_add(
                            out=u_sb[:, 2:4, :], in0=s2_sb[:, 4:6, :], in1=s2_sb[:, 6:8, :]
                        )
                        vector.tensor_add(
                            out=s2_sb[:, 0:2, :], in0=u_sb[:, 0:2, :], in1=u_sb[:, 2:4, :]
                        )
                        vector.tensor_add(
                            out=s1_sb[:, sb1, :], in0=s2_sb[:, 0, :], in1=s2_sb[:, 1, :]
                        ).then_inc(tree_sem, 1)
                    # ctxT PSUM -> SBUF bf16 staging copy (the last chunk's
                    # copy is done by the scalar engine instead)
                    if not last:
                        vector.wait_ge(pv_sem, hq + 1)
                        if hq >= 2:
                            vector.wait_ge(st_sems[hq % 2], 16 * (hq // 2))
                        vector.tensor_scalar_add(
                            out=out_sb[:, cb, :],
                            in0=psum_cx[:, cb * QC : (cb + 1) * QC],
                            scalar1=0.0,
                        ).then_inc(cp_sem, 1)

        @block.gpsimd
        def _(gpsimd):
            for hq in range(NHQ):
                h, qc = divmod(hq, NQC)
                gpsimd.wait_ge(tree_sem, hq + 1)
                gpsimd.dma_start(
                    out=s_out[h, qc], in_=s1_sb[:, hq % 2, :]
                ).then_inc(s1st_sems[hq % 2], 16)
            gpsimd.wait_ge(s1st_sems[0], 16 * (NHQ // 2))
            gpsimd.wait_ge(s1st_sems[1], 16 * (NHQ // 2))

    return nc


def _perm_blocks(x, idx):
    xb = x.reshape(B, S // BLOCK, BLOCK, H, D)
    return xb[:, idx].reshape(B, S, H, D)


def kernel(**inputs):
    from concourse.bass_utils import run_bass_kernel_spmd

    q = np.asarray(inputs["query"], dtype=np.float32)
    k = np.asarray(inputs["key"], dtype=np.float32)
    v = np.asarray(inputs["value"], dtype=np.float32)
    hp = np.asarray(inputs["head_perm_idx"]).astype(np.int64)
    hd = np.asarray(inputs["head_deperm_idx"]).astype(np.int64)
    rp = np.asarray(inputs["new_row_perm_idx"]).astype(np.int64)
    cp = np.asarray(inputs["new_col_perm_idx"]).astype(np.int64)
    rd = np.asarray(inputs["new_row_deperm_idx"]).astype(np.int64)

    qp = _perm_blocks(q[:, :, hp], rp)[0]  # [S, H, D]
    kp = _perm_blocks(k[:, :, hp], cp)[0]
    vp = _perm_blocks(v[:, :, hp], cp)[0]

    bf = ml_dtypes.bfloat16
    qt = np.ascontiguousarray(qp.transpose(1, 2, 0)).astype(bf)  # [H, D, S]
    kt = np.ascontiguousarray(kp.transpose(1, 2, 0)).astype(bf)  # [H, D, S]
    # va[h, kp, kt, d] = V[h, kt*128 + kp, d]
    va = np.ascontiguousarray(
        vp.transpose(1, 0, 2).reshape(H, NT, 128, D).transpose(0, 2, 1, 3)
    ).astype(bf)

    if "nc" not in _CACHE:
        _CACHE["nc"] = _build_bass()
    nc = _CACHE["nc"]

    core_ids = list(range(NCORES))
    in_maps = [
        {
            "kt_in": np.ascontiguousarray(kt[c * HPC : (c + 1) * HPC]),
            "qt_in": np.ascontiguousarray(qt[c * HPC : (c + 1) * HPC]),
            "va_in": np.ascontiguousarray(va[c * HPC : (c + 1) * HPC]),
            "kq0_in": np.ascontiguousarray(
                np.concatenate(
                    [qt[c * HPC][:, 0:QC], kt[c * HPC][:, 0:1536]], axis=1
                )
            ),
        }
        for c in core_ids
    ]
    res = run_bass_kernel_spmd(nc, in_maps, core_ids)
    _CACHE["last_result"] = res

    ctxT = np.concatenate(
        [np.asarray(res.results[c]["ctx_out"], dtype=np.float32) for c in core_ids],
        axis=0,
    )  # [H, D, S] fp32 (from bf16), unnormalized
    s1 = np.concatenate(
        [np.asarray(res.results[c]["s_out"], dtype=np.float32) for c in core_ids],
        axis=0,
    )  # [H, NQC, 128, QC]
    denom = s1.sum(axis=2).reshape(H, S)  # [H, S]
    ctxT = ctxT / denom[:, None, :]
    ctx = np.ascontiguousarray(ctxT.transpose(2, 0, 1))[None]  # [1, S, H, D]
    ctx = _perm_blocks(ctx, rd)
    out = ctx[:, :, hd]
    return np.ascontiguousarray(out, dtype=np.float32)


# revision 84
# speedup vs baseline: 1.0707x; 1.0473x over previous
"""Trainium2 Bass kernel for LongContextAttention (B=1, S=2048, H=16, D=128).

Strategy: permutations are pure data movement -> host-side numpy gathers.
Attention is head-parallel: 16 heads / 8 cores = 2 heads per core.

Per core, software-pipelined over 128 (h, q-chunk, k-tile) steps. The wall
clock decomposes as startup (~12us: framework preamble + first input DMA) +
a saturated ScalarE exp stream (~61us) + drain tail (~7us: last PV + fold +
store receipts), so the design keeps the exp stream gapless, offloads 1/16
of the exp work to the under-used DVE, and squeezes the two ends:
  - QK^T transposed: scoresT[k,q] = sum_d KT[d,k] QT[d,q]; stationary = KT
    tile (128x128), moving = QT chunk (128x512). PSUM banks rotate globally
    (seq%6, 6 banks) so the PE always runs ~6 k-tiles ahead of the exp.
  - exp on ScalarE for 15 of 16 k-tiles per chunk, in groups [3,3,3,3,2,1]
    so every group reads a contiguous, non-wrapping PSUM span for both bank
    phases (15*hq % 6 cycles 0,3); larger spans would stall at group
    boundaries (only 6-n banks of lookahead). One qk_sem wait per group plus
    one cp_sem pt-reuse wait per chunk keeps the scalar queue lean.
  - k-tile 14 of each chunk is exp'd on the DVE instead, via the Schraudolph
    fast-exp (int16(score*A+B) bit-cast as bf16, ~1.8% rms -> ~0.45% output
    error vs the 2e-2 budget). Its QK lands in the idle psum_cx spare bank
    (the one PV ping-pongs away from), so the slower DVE op never holds up
    the 6-bank rotation; dexp_sem guards the spare bank against the next
    chunk's PV and gates PV of tile 14 itself.
  - PE warm-up: 5 dummy matmuls at block entry push the HAM clock gate
    toward 2.4 GHz before the first real QK, and finish before its data
    lands so they add no latency.
  - PV with V as STATIONARY: psum_ctxT[d, q-chunk] accumulates over the 16
    k-tiles, moving = pt[:, kt, qchunk] (512 cols), 2 PSUM banks ping-pong.
  - denominator: 3-phase binary slab fold on DVE per chunk (8 tensor_adds,
    mostly FD=1024) phased inside that chunk's own exp window; the last
    chunk pre-folds planes 0:13 so only 2 FD=512 adds trail the final exp.
    s1[k, q] bf16 partials; the final 128-partition sum happens on the host.
  - ctxT PSUM -> SBUF copy on DVE converts to bf16; DMA out bf16 (halves the
    tail store). Host divides by the denominator and transposes (device
    output is ctx^T [h, d, q]).
  - input DMA: SDMA round-robins in-flight transfers at packet granularity,
    so the urgent slices are small/first (kq0 slab = qt chunk0 ++ kt tiles
    0:12, one contiguous DRAM source for large per-partition descriptors)
    and bulky loads are gated on pipeline progress (va0 split 4/12 tiles
    ungated, head-1 loads behind exp_sem >= 10) to keep them off the
    critical path without starving the in-order tensor engine's PV waits.
"""

import numpy as np
import ml_dtypes

B, S, H, D = 1, 2048, 16, 128
BLOCK = 64
NCORES = 8
HPC = H // NCORES          # heads per core = 2
NT = S // 128              # 16 k-tiles
NQC = 4                    # q-chunks of 512
QC = 512
SCALE = 1.0 / float(np.sqrt(D))
NSTEP = HPC * NQC * NT     # 128 (h, qc, kt) steps
NHQ = HPC * NQC            # 8 (h, qc) chunks
NQKBUF = 6                 # PSUM banks rotating for QK output

# Per chunk, k-tiles 14 AND 15 are offloaded to the DVE (fast-exp via the
# Schraudolph int16-bitcast trick); their QK outputs cycle through the idle
# psum_cx spare bank so the 6-bank rotation never waits on the slower DVE.
# ScalarE handles tiles 0..13 in wrap-free groups chosen per bank phase
# (14*hq % 6 cycles 0,2,4).
DVE_KT = 14                # first offloaded k-tile (14 and 15)
SPC = NT - 2               # scalar tiles per chunk = 14
PATTERNS = {0: [3, 3, 3, 3, 2], 2: [3, 1, 3, 3, 3, 1], 4: [2, 3, 3, 3, 3]}


def _pattern(hq):
    return PATTERNS[(SPC * hq) % NQKBUF]


# GRPS entries: (seq0, n, hq, kt0, ord) where seq0 is the chunk-global
# scalar sequence position (bank = seq % 6), kt0 the group's first k-tile
# (scalar tiles are 0..13 so kt == scalar index), ord the group's ordinal
# within its chunk.
GRPS = []
for _hq in range(NHQ):
    _s = 0
    for _o, _n in enumerate(_pattern(_hq)):
        _seq0 = SPC * _hq + _s
        assert (_seq0 % NQKBUF) + _n <= NQKBUF
        GRPS.append((_seq0, _n, _hq, _s, _o))
        _s += _n
    assert _s == SPC
GRPS_END = [0] * NHQ       # number of groups up to and including hq
for _gi, _g in enumerate(GRPS):
    GRPS_END[_g[2]] = max(GRPS_END[_g[2]], _gi + 1)

# Schraudolph fast-exp constants: i16 = round(score*A_FEXP + B_FEXP)
# bit-cast as bf16 equals exp(score*SCALE) with ~1.8% rms relative error
# (1/16 of all tiles -> ~0.45% output contribution; budget is 2e-2).
A_FEXP = float(SCALE * 128.0 / np.log(2.0))
B_FEXP = 127.0 * 128.0 - 7.375

# per-chunk fold phases: group count (within the chunk) after which >= 8 /
# >= 12 of the 16 k-planes are available.
FOLD_P1 = [0] * NHQ  # exp_sem value enabling planes 0:8
FOLD_P2 = [0] * NHQ  # exp_sem value enabling planes 0:12
FOLD_P2B = [0] * NHQ  # exp_sem value enabling planes 0:14
for _hq in range(NHQ):
    _cum = 0
    _base = GRPS_END[_hq - 1] if _hq > 0 else 0
    for _i, _n in enumerate(_pattern(_hq)):
        _cum += _n
        if FOLD_P1[_hq] == 0 and _cum >= 8:
            FOLD_P1[_hq] = _base + _i + 1
        if FOLD_P2[_hq] == 0 and _cum >= 12:
            FOLD_P2[_hq] = _base + _i + 1
        if FOLD_P2B[_hq] == 0 and _cum >= 14:
            FOLD_P2B[_hq] = _base + _i + 1

_CACHE = {}


def _build_bass():
    import concourse.bass as bass
    import concourse.mybir as mybir
    from contextlib import ExitStack

    f32 = mybir.dt.float32
    bf16 = mybir.dt.bfloat16

    nc = bass.Bass()
    kt_in = nc.declare_dram_parameter("kt_in", [HPC, D, S], bf16, isOutput=False)
    qt_in = nc.declare_dram_parameter("qt_in", [HPC, D, S], bf16, isOutput=False)
    va_in = nc.declare_dram_parameter("va_in", [HPC, 128, NT, D], bf16, isOutput=False)
    kq0_in = nc.declare_dram_parameter("kq0_in", [128, 2048], bf16, isOutput=False)
    ctx_out = nc.declare_dram_parameter("ctx_out", [HPC, D, S], bf16, isOutput=True)
    s_out = nc.declare_dram_parameter("s_out", [HPC, NQC, 128, QC], bf16, isOutput=True)

    ctx = ExitStack()
    with ctx:
        kt_sb = ctx.enter_context(nc.sbuf_tensor("kt_sb", [128, HPC, S], bf16))
        qt_sb = ctx.enter_context(nc.sbuf_tensor("qt_sb", [128, HPC, S], bf16))
        # prologue staging: qt0[:, 0:512] ++ kt0[:, 0:1536] (one DRAM slab ->
        # large per-partition descriptors -> fastest first-byte path)
        kq0_sb = ctx.enter_context(nc.sbuf_tensor("kq0_sb", [128, 2048], bf16))
        va_sb = ctx.enter_context(nc.sbuf_tensor("va_sb", [128, HPC, NT, D], bf16))
        pt_sb = [
            ctx.enter_context(nc.sbuf_tensor(f"pt_sb{i}", [128, NT, QC], bf16))
            for i in range(3)
        ]
        s2_sb = ctx.enter_context(nc.sbuf_tensor("s2_sb", [128, 8, QC], bf16))
        u_sb = ctx.enter_context(nc.sbuf_tensor("u_sb", [128, 4, QC], bf16))
        s1_sb = ctx.enter_context(nc.sbuf_tensor("s1_sb", [128, 2, QC], bf16))
        out_sb = ctx.enter_context(nc.sbuf_tensor("out_sb", [128, 2, QC], bf16))
        warm_sb = ctx.enter_context(nc.sbuf_tensor("warm_sb", [128, 2], f32))

        psum_qk = ctx.enter_context(nc.psum_tensor("psum_qk", [128, NQKBUF * QC], f32))
        psum_cx = ctx.enter_context(nc.psum_tensor("psum_cx", [128, 2 * QC], f32))

        load_sems = [ctx.enter_context(nc.semaphore(f"load_sem{i}")) for i in range(10)]
        qk_sem = ctx.enter_context(nc.semaphore("qk_sem"))
        exp_sem = ctx.enter_context(nc.semaphore("exp_sem"))
        sqk_sem = ctx.enter_context(nc.semaphore("sqk_sem"))
        dexp_sem = ctx.enter_context(nc.semaphore("dexp_sem"))
        pv_sem = ctx.enter_context(nc.semaphore("pv_sem"))
        tree_sem = ctx.enter_context(nc.semaphore("tree_sem"))
        cp_sem = ctx.enter_context(nc.semaphore("cp_sem"))
        st_sems = [ctx.enter_context(nc.semaphore(f"st_sem{i}")) for i in range(2)]
        s1st_sems = [ctx.enter_context(nc.semaphore(f"s1st_sem{i}")) for i in range(2)]

        block = ctx.enter_context(nc.Block(no_gpsimd_drain=True))

        @block.sync
        def _(sync):
            # Priority-phased input loads on the HWDGE (sync) path. The SDMA
            # engines round-robin between all queued transfers at packet
            # granularity, so lower-priority loads are gated behind qk_sem
            # progress to give the critical slices exclusive bandwidth.
            # Most urgent first: the staging slab slice QK(0..2) needs, then
            # the rest of the slab (tiles 3:12). Packet-level round-robin
            # favors small in-flight transfers, and the bulky va0/head-1
            # loads are additionally gated on pipeline progress.
            sync.dma_start(out=kq0_sb[:, 0:896], in_=kq0_in[:, 0:896]).then_inc(
                load_sems[0], 16
            )
            sync.dma_start(out=kq0_sb[:, 896:2048], in_=kq0_in[:, 896:2048]).then_inc(
                load_sems[7], 16
            )
            sync.dma_start(out=kt_sb[:, 0, 1536:S], in_=kt_in[0][:, 1536:S]).then_inc(
                load_sems[1], 16
            )
            sync.dma_start(out=qt_sb[:, 0, QC:S], in_=qt_in[0][:, QC:S]).then_inc(
                load_sems[3], 16
            )
            sync.dma_start(out=va_sb[:, 0, 0:4, :], in_=va_in[0][:, 0:4, :]).then_inc(
                load_sems[2], 16
            )
            sync.dma_start(out=va_sb[:, 0, 4:NT, :], in_=va_in[0][:, 4:NT, :]).then_inc(
                load_sems[8], 16
            )
            sync.wait_ge(exp_sem, 10)      # head-1 loads have ~30us of slack
            sync.dma_start(out=kt_sb[:, 1, :], in_=kt_in[1]).then_inc(load_sems[4], 16)
            sync.dma_start(out=qt_sb[:, 1, :], in_=qt_in[1]).then_inc(load_sems[6], 16)
            sync.dma_start(out=va_sb[:, 1, :, :], in_=va_in[1]).then_inc(load_sems[5], 16)
            # Output ctx DMAs from here (sync is idle after the loads);
            # the s1 DMAs stay on gpsimd so the triggers run in parallel.
            for hq in range(NHQ):
                h, qc = divmod(hq, NQC)
                sync.wait_ge(cp_sem, hq + 1)
                sync.dma_start(
                    out=ctx_out[h][:, qc * QC : (qc + 1) * QC],
                    in_=out_sb[:, hq % 2, :],
                ).then_inc(st_sems[hq % 2], 16)
            sync.wait_ge(st_sems[0], 16 * (NHQ // 2))
            sync.wait_ge(st_sems[1], 16 * (NHQ // 2))

        @block.tensor
        def _(tensor):
            def qk_operands(hq, kt):
                h, qc = divmod(hq, NQC)
                if h == 0 and kt < 12:
                    lhs = kq0_sb[:, 512 + kt * 128 : 512 + (kt + 1) * 128]
                else:
                    lhs = kt_sb[:, h, kt * 128 : (kt + 1) * 128]
                if h == 0 and qc == 0:
                    rhs = kq0_sb[:, 0:512]
                else:
                    rhs = qt_sb[:, h, qc * QC : (qc + 1) * QC]
                return lhs, rhs

            def emit_qk(seq):
                hq, kt = divmod(seq, SPC)
                if seq == 0:
                    tensor.wait_ge(load_sems[0], 16)   # kq0 first slice (qt c0 + kt 0:3)
                if seq == 3:
                    tensor.wait_ge(load_sems[7], 16)   # kq0 rest (kt 3:12)
                if seq == 12:
                    tensor.wait_ge(load_sems[1], 16)   # kt0 tiles 12:16
                if seq == SPC:
                    tensor.wait_ge(load_sems[3], 16)   # qt0 rest
                if seq == NQC * SPC:
                    tensor.wait_ge(load_sems[4], 16)   # kt1
                    tensor.wait_ge(load_sems[6], 16)   # qt1
                lhs, rhs = qk_operands(hq, kt)
                b = seq % NQKBUF
                tensor.matmul(
                    psum_qk[:, b * QC : (b + 1) * QC],
                    lhs,
                    rhs,
                    start=True,
                    stop=True,
                ).then_inc(qk_sem, 1)

            def emit_spare_qk(hq, kt):
                # an offloaded tile's QK goes to the idle psum_cx spare bank
                if kt == DVE_KT:
                    if hq >= 1:
                        tensor.wait_ge(cp_sem, hq)       # spare bank drained
                else:
                    tensor.wait_ge(dexp_sem, 2 * hq + 1)  # tile-14 TS read out
                lhs, rhs = qk_operands(hq, kt)
                sp = (hq + 1) % 2
                tensor.matmul(
                    psum_cx[:, sp * QC : (sp + 1) * QC],
                    lhs,
                    rhs,
                    start=True,
                    stop=True,
                ).then_inc(sqk_sem, 1)

            def emit_pv(hq2, kt2):
                h2 = hq2 // NQC
                cb = hq2 % 2
                pb2 = hq2 % 3
                if kt2 == 0:
                    tensor.wait_ge(load_sems[2] if h2 == 0 else load_sems[5], 16)
                    if hq2 >= 2:
                        tensor.wait_ge(cp_sem, hq2 - 1)  # psum_cx buf drained
                    if hq2 >= 1:
                        tensor.wait_ge(dexp_sem, 2 * hq2)  # spare bank read out
                if kt2 == 4 and hq2 == 0:
                    tensor.wait_ge(load_sems[8], 16)     # va0 tiles 4:16
                if kt2 >= DVE_KT:
                    tensor.wait_ge(dexp_sem, 2 * hq2 + (kt2 - DVE_KT) + 1)
                mm = tensor.matmul(
                    psum_cx[:, cb * QC : (cb + 1) * QC],
                    va_sb[:, h2, kt2, :],
                    pt_sb[pb2][:, kt2, :],
                    start=(kt2 == 0),
                    stop=(kt2 == NT - 1),
                )
                if kt2 == NT - 1:
                    mm.then_inc(pv_sem, 1)

            # Warm the PE HAM clock gate (1.2 -> 2.4 GHz needs ~3.4us of
            # sustained activity): dummy matmuls on resident SBUF garbage
            # into psum bank 0, overwritten later by QK g=0 (start=True).
            for _ in range(5):
                tensor.matmul(
                    psum_qk[:, 0:QC],
                    kq0_sb[:, 0:128],
                    kq0_sb[:, 0:QC],
                    start=True,
                    stop=True,
                )
            for seq in range(NQKBUF):
                emit_qk(seq)
            for gi, (s0, n, hq, kt0, o) in enumerate(GRPS):
                tensor.wait_ge(exp_sem, gi + 1)
                for seq in range(s0 + NQKBUF, min(s0 + NQKBUF + n, NHQ * SPC)):
                    emit_qk(seq)
                if o == 2:
                    emit_spare_qk(hq, DVE_KT)
                if o == len(_pattern(hq)) - 1:
                    emit_spare_qk(hq, DVE_KT + 1)
                if o == 0 and hq >= 1:
                    # previous chunk's offloaded-tile PVs, deferred here so
                    # their dexp waits never block that chunk's QK refills
                    emit_pv(hq - 1, DVE_KT)
                    emit_pv(hq - 1, DVE_KT + 1)
                for j in range(n):
                    emit_pv(hq, kt0 + j)
            emit_pv(NHQ - 1, DVE_KT)
            emit_pv(NHQ - 1, DVE_KT + 1)

        @block.scalar
        def _(scalar):
            import concourse.mybir as mybir_

            # Warm the Exp activation table immediately: the input is a
            # preamble-initialized const AP, so no waits and no memset chain.
            scalar.activation(
                out=warm_sb[:, 1:2],
                in_=nc.const_aps.tensor(0.0, (128, 1), mybir_.dt.float32),
                func=mybir_.ActivationFunctionType.Exp,
                scale=1.0,
            )
            for gi, (s0, n, hq, kt0, o) in enumerate(GRPS):
                if kt0 == 0 and hq >= 3:
                    # pt buf reused from (hq-3): cp_sem (the ctx copy) implies
                    # both PV and the denominator fold are done
                    scalar.wait_ge(cp_sem, hq - 2)
                scalar.wait_ge(qk_sem, s0 + n)
                b0 = s0 % NQKBUF
                scalar.activation(
                    out=pt_sb[hq % 3][:, kt0 : kt0 + n, :],
                    in_=psum_qk[:, b0 * QC : (b0 + n) * QC],
                    func=mybir_.ActivationFunctionType.Exp,
                    scale=SCALE,
                ).then_inc(exp_sem, 1)
            # The last chunk's ctxT copy runs here (scalar is idle after its
            # final exp; Copy shares the exp table set, so no table reload)
            # in parallel with the final denominator fold on the DVE.
            scalar.wait_ge(pv_sem, NHQ)
            scalar.wait_ge(st_sems[(NHQ - 1) % 2], 16 * ((NHQ - 1) // 2))
            scalar.copy(
                out=out_sb[:, (NHQ - 1) % 2, :],
                in_=psum_cx[:, ((NHQ - 1) % 2) * QC : ((NHQ - 1) % 2 + 1) * QC],
            ).then_inc(cp_sem, 1)

        @block.vector
        def _(vector):
            # Denominator: per chunk, a binary slab fold of the 16 exp planes
            # (mostly FD=1024 tensor_adds), phased in 3 steps so the bulk
            # overlaps the chunk's own exp stream:
            #   P1 (>=8 planes):  a = p0:2+p2:4 -> s2[0:2]; b = p4:6+p6:8 -> s2[2:4]
            #   P2 (>=12 planes): c = p8:10+p10:12 -> s2[4:6]; u0 = a+b -> u[0:2]
            #   P3 (all 16):      d = p12:14+p14:16 -> s2[6:8]; u1 = c+d -> u[2:4];
            #                     w = u0+u1 -> s2[0:2]; s1 = w0+w1
            # then the ctxT PSUM -> SBUF (bf16) staging copy.
            for hq in range(NHQ):
                pb = hq % 3
                sb1 = hq % 2
                cb = hq % 2
                last = hq == NHQ - 1
                pt = pt_sb[pb]
                # this chunk's offloaded k-tiles 14/15: fast-exp on the DVE
                # from the psum_cx spare bank (outside the 6-bank rotation)
                sp = (hq + 1) % 2

                def dve_exp(dk):
                    vector.wait_ge(sqk_sem, 2 * hq + (dk - DVE_KT) + 1)
                    vector.tensor_scalar(
                        out=pt[:, dk : dk + 1, :].bitcast(mybir.dt.int16),
                        in0=psum_cx[:, sp * QC : (sp + 1) * QC],
                        scalar1=A_FEXP,
                        scalar2=B_FEXP,
                        op0=mybir.AluOpType.mult,
                        op1=mybir.AluOpType.add,
                    ).then_inc(dexp_sem, 1)

                dve_exp(DVE_KT)
                with nc.allow_low_precision("bf16 partials; host fp32 final sum"):
                    vector.wait_ge(exp_sem, FOLD_P1[hq])
                    vector.tensor_add(
                        out=s2_sb[:, 0:2, :], in0=pt[:, 0:2, :], in1=pt[:, 2:4, :]
                    )
                    vector.tensor_add(
                        out=s2_sb[:, 2:4, :], in0=pt[:, 4:6, :], in1=pt[:, 6:8, :]
                    )
                    vector.wait_ge(exp_sem, FOLD_P2[hq])
                    vector.tensor_add(
                        out=s2_sb[:, 4:6, :], in0=pt[:, 8:10, :], in1=pt[:, 10:12, :]
                    )
                    vector.tensor_add(
                        out=u_sb[:, 0:2, :], in0=s2_sb[:, 0:2, :], in1=s2_sb[:, 2:4, :]
                    )
                    if last:
                        # pre-compute m = p0..11 and t = p14+p15 (DVE planes,
                        # ready mid-window) so only 3 small FD=512 adds remain
                        # after the final exp group: d1 = p12+p13; q = m+d1;
                        # s1 = q+t
                        vector.tensor_add(
                            out=u_sb[:, 3, :], in0=s2_sb[:, 4, :], in1=s2_sb[:, 5, :]
                        )
                        vector.tensor_add(
                            out=s2_sb[:, 6, :], in0=u_sb[:, 0, :], in1=u_sb[:, 1, :]
                        )
                        vector.tensor_add(
                            out=s2_sb[:, 5, :], in0=u_sb[:, 3, :], in1=s2_sb[:, 6, :]
                        )
                    # tile-15 fast-exp here (its spare QK is emitted near the
                    # chunk's last group, after the tile-14 TS freed the bank)
                    dve_exp(DVE_KT + 1)
                    if last:
                        vector.tensor_add(
                            out=s2_sb[:, 7, :], in0=pt[:, 14, :], in1=pt[:, 15, :]
                        )
                    # final fold phase first: it only needs the chunk's last
                    # exp group (~3us before PV+copy), so the s1 store never
                    # backs up behind the pv-gated ctx copy.
                    vector.wait_ge(exp_sem, GRPS_END[hq])
                    if hq >= 2:
                        vector.wait_ge(s1st_sems[hq % 2], 16 * (hq // 2))
                    if last:
                        vector.tensor_add(
                            out=u_sb[:, 2, :], in0=pt[:, 12, :], in1=pt[:, 13, :]
                        )
                        vector.tensor_add(
                            out=s2_sb[:, 4, :], in0=u_sb[:, 2, :], in1=s2_sb[:, 5, :]
                        )
                        vector.tensor_add(
                            out=s1_sb[:, sb1, :], in0=s2_sb[:, 4, :], in1=s2_sb[:, 7, :]
                        ).then_inc(tree_sem, 1)
                    else:
                        vector.tensor_add(
                            out=s2_sb[:, 6:8, :], in0=pt[:, 12:14, :], in1=pt[:, 14:16, :]
                        )
                        vector.tensor_add(
                            out=u_sb[:, 2:4, :], in0=s2_sb[:, 4:6, :], in1=s2_sb[:, 6:8, :]
                        )
                        vector.tensor_add(
                            out=s2_sb[:, 0:2, :], in0=u_sb[:, 0:2, :], in1=u_sb[:, 2:4, :]
                        )
                        vector.tensor_add(
                            out=s1_sb[:, sb1, :], in0=s2_sb[:, 0, :], in1=s2_sb[:, 1, :]
                        ).then_inc(tree_sem, 1)
                    # ctxT PSUM -> SBUF bf16 staging copy (the last chunk's
                    # copy is done by the scalar engine instead)
                    if not last:
                        vector.wait_ge(pv_sem, hq + 1)
                        if hq >= 2:
                            vector.wait_ge(st_sems[hq % 2], 16 * (hq // 2))
                        vector.tensor_scalar_add(
                            out=out_sb[:, cb, :],
                            in0=psum_cx[:, cb * QC : (cb + 1) * QC],
                            scalar1=0.0,
                        ).then_inc(cp_sem, 1)

        @block.gpsimd
        def _(gpsimd):
            for hq in range(NHQ):
                h, qc = divmod(hq, NQC)
                gpsimd.wait_ge(tree_sem, hq + 1)
                gpsimd.dma_start(
                    out=s_out[h, qc], in_=s1_sb[:, hq % 2, :]
                ).then_inc(s1st_sems[hq % 2], 16)
            gpsimd.wait_ge(s1st_sems[0], 16 * (NHQ // 2))
            gpsimd.wait_ge(s1st_sems[1], 16 * (NHQ // 2))

    return nc


def _perm_blocks(x, idx):
    xb = x.reshape(B, S // BLOCK, BLOCK, H, D)
    return xb[:, idx].reshape(B, S, H, D)


def kernel(**inputs):
    from concourse.bass_utils import run_bass_kernel_spmd

    q = np.asarray(inputs["query"], dtype=np.float32)
    k = np.asarray(inputs["key"], dtype=np.float32)
    v = np.asarray(inputs["value"], dtype=np.float32)
    hp = np.asarray(inputs["head_perm_idx"]).astype(np.int64)
    hd = np.asarray(inputs["head_deperm_idx"]).astype(np.int64)
    rp = np.asarray(inputs["new_row_perm_idx"]).astype(np.int64)
    cp = np.asarray(inputs["new_col_perm_idx"]).astype(np.int64)
    rd = np.asarray(inputs["new_row_deperm_idx"]).astype(np.int64)

    qp = _perm_blocks(q[:, :, hp], rp)[0]  # [S, H, D]
    kp = _perm_blocks(k[:, :, hp], cp)[0]
    vp = _perm_blocks(v[:, :, hp], cp)[0]

    bf = ml_dtypes.bfloat16
    qt = np.ascontiguousarray(qp.transpose(1, 2, 0)).astype(bf)  # [H, D, S]
    kt = np.ascontiguousarray(kp.transpose(1, 2, 0)).astype(bf)  # [H, D, S]
    # va[h, kp, kt, d] = V[h, kt*128 + kp, d]
    va = np.ascontiguousarray(
        vp.transpose(1, 0, 2).reshape(H, NT, 128, D).transpose(0, 2, 1, 3)
    ).astype(bf)

    if "nc" not in _CACHE:
        _CACHE["nc"] = _build_bass()
    nc = _CACHE["nc"]

    core_ids = list(range(NCORES))
    in_maps = [
        {
            "kt_in": np.ascontiguousarray(kt[c * HPC : (c + 1) * HPC]),
            "qt_in": np.ascontiguousarray(qt[c * HPC : (c + 1) * HPC]),
            "va_in": np.ascontiguousarray(va[c * HPC : (c + 1) * HPC]),
            "kq0_in": np.ascontiguousarray(
                np.concatenate(
                    [qt[c * HPC][:, 0:QC], kt[c * HPC][:, 0:1536]], axis=1
                )
            ),
        }
        for c in core_ids
    ]
    res = run_bass_kernel_spmd(nc, in_maps, core_ids)
    _CACHE["last_result"] = res

    ctxT = np.concatenate(
        [np.asarray(res.results[c]["ctx_out"], dtype=np.float32) for c in core_ids],
        axis=0,
    )  # [H, D, S] fp32 (from bf16), unnormalized
    s1 = np.concatenate(
        [np.asarray(res.results[c]["s_out"], dtype=np.float32) for c in core_ids],
        axis=0,
    )  # [H, NQC, 128, QC]
    denom = s1.sum(axis=2).reshape(H, S)  # [H, S]
    ctxT = ctxT / denom[:, None, :]
    ctx = np.ascontiguousarray(ctxT.transpose(2, 0, 1))[None]  # [1, S, H, D]
    ctx = _perm_blocks(ctx, rd)
    out = ctx[:, :, hd]
    return np.ascontiguousarray(out, dtype=np.float32)


# revision 87
# speedup vs baseline: 1.0762x; 1.0051x over previous
"""Trainium2 Bass kernel for LongContextAttention (B=1, S=2048, H=16, D=128).

Strategy: permutations are pure data movement -> host-side numpy gathers.
Attention is head-parallel: 16 heads / 8 cores = 2 heads per core.

Per core, software-pipelined over 128 (h, q-chunk, k-tile) steps. The wall
clock decomposes as startup (~12us: framework preamble + first input DMA) +
a saturated ScalarE exp stream (~61us) + drain tail (~7us: last PV + fold +
store receipts), so the design keeps the exp stream gapless, offloads 1/16
of the exp work to the under-used DVE, and squeezes the two ends:
  - QK^T transposed: scoresT[k,q] = sum_d KT[d,k] QT[d,q]; stationary = KT
    tile (128x128), moving = QT chunk (128x512). PSUM banks rotate globally
    (seq%6, 6 banks) so the PE always runs ~6 k-tiles ahead of the exp.
  - exp on ScalarE for 15 of 16 k-tiles per chunk, in groups [3,3,3,3,2,1]
    so every group reads a contiguous, non-wrapping PSUM span for both bank
    phases (15*hq % 6 cycles 0,3); larger spans would stall at group
    boundaries (only 6-n banks of lookahead). One qk_sem wait per group plus
    one cp_sem pt-reuse wait per chunk keeps the scalar queue lean.
  - k-tile 14 of each chunk is exp'd on the DVE instead, via the Schraudolph
    fast-exp (int16(score*A+B) bit-cast as bf16, ~1.8% rms -> ~0.45% output
    error vs the 2e-2 budget). Its QK lands in the idle psum_cx spare bank
    (the one PV ping-pongs away from), so the slower DVE op never holds up
    the 6-bank rotation; dexp_sem guards the spare bank against the next
    chunk's PV and gates PV of tile 14 itself.
  - PE warm-up: 5 dummy matmuls at block entry push the HAM clock gate
    toward 2.4 GHz before the first real QK, and finish before its data
    lands so they add no latency.
  - PV with V as STATIONARY: psum_ctxT[d, q-chunk] accumulates over the 16
    k-tiles, moving = pt[:, kt, qchunk] (512 cols), 2 PSUM banks ping-pong.
  - denominator: 3-phase binary slab fold on DVE per chunk (8 tensor_adds,
    mostly FD=1024) phased inside that chunk's own exp window; the last
    chunk pre-folds planes 0:13 so only 2 FD=512 adds trail the final exp.
    s1[k, q] bf16 partials; the final 128-partition sum happens on the host.
  - ctxT PSUM -> SBUF copy on DVE converts to bf16; DMA out bf16 (halves the
    tail store). Host divides by the denominator and transposes (device
    output is ctx^T [h, d, q]).
  - input DMA: SDMA round-robins in-flight transfers at packet granularity,
    so the urgent slices are small/first (kq0 slab = qt chunk0 ++ kt tiles
    0:12, one contiguous DRAM source for large per-partition descriptors)
    and bulky loads are gated on pipeline progress (va0 split 4/12 tiles
    ungated, head-1 loads behind exp_sem >= 10) to keep them off the
    critical path without starving the in-order tensor engine's PV waits.
"""

import numpy as np
import ml_dtypes

B, S, H, D = 1, 2048, 16, 128
BLOCK = 64
NCORES = 8
HPC = H // NCORES          # heads per core = 2
NT = S // 128              # 16 k-tiles
NQC = 4                    # q-chunks of 512
QC = 512
SCALE = 1.0 / float(np.sqrt(D))
NSTEP = HPC * NQC * NT     # 128 (h, qc, kt) steps
NHQ = HPC * NQC            # 8 (h, qc) chunks
NQKBUF = 6                 # PSUM banks rotating for QK output

# Per chunk, k-tiles 14 AND 15 are offloaded to the DVE (fast-exp via the
# Schraudolph int16-bitcast trick); their QK outputs cycle through the idle
# psum_cx spare bank so the 6-bank rotation never waits on the slower DVE.
# ScalarE handles tiles 0..13 in wrap-free groups chosen per bank phase
# (14*hq % 6 cycles 0,2,4).
DVE_KT = 14                # first offloaded k-tile (14 and 15)
SPC = NT - 2               # scalar tiles per chunk = 14
PATTERNS = {0: [3, 3, 3, 3, 2], 2: [3, 1, 3, 3, 3, 1], 4: [2, 3, 3, 3, 3]}


def _pattern(hq):
    return PATTERNS[(SPC * hq) % NQKBUF]


# GRPS entries: (seq0, n, hq, kt0, ord) where seq0 is the chunk-global
# scalar sequence position (bank = seq % 6), kt0 the group's first k-tile
# (scalar tiles are 0..13 so kt == scalar index), ord the group's ordinal
# within its chunk.
GRPS = []
for _hq in range(NHQ):
    _s = 0
    for _o, _n in enumerate(_pattern(_hq)):
        _seq0 = SPC * _hq + _s
        assert (_seq0 % NQKBUF) + _n <= NQKBUF
        GRPS.append((_seq0, _n, _hq, _s, _o))
        _s += _n
    assert _s == SPC
GRPS_END = [0] * NHQ       # number of groups up to and including hq
for _gi, _g in enumerate(GRPS):
    GRPS_END[_g[2]] = max(GRPS_END[_g[2]], _gi + 1)

# Schraudolph fast-exp constants: i16 = round(score*A_FEXP + B_FEXP)
# bit-cast as bf16 equals exp(score*SCALE) with ~1.8% rms relative error
# (1/16 of all tiles -> ~0.45% output contribution; budget is 2e-2).
A_FEXP = float(SCALE * 128.0 / np.log(2.0))
B_FEXP = 127.0 * 128.0 - 7.375

# per-chunk fold phases: group count (within the chunk) after which >= 8 /
# >= 12 of the 16 k-planes are available.
FOLD_P1 = [0] * NHQ  # exp_sem value enabling planes 0:8
FOLD_P2 = [0] * NHQ  # exp_sem value enabling planes 0:12
FOLD_P2B = [0] * NHQ  # exp_sem value enabling planes 0:14
for _hq in range(NHQ):
    _cum = 0
    _base = GRPS_END[_hq - 1] if _hq > 0 else 0
    for _i, _n in enumerate(_pattern(_hq)):
        _cum += _n
        if FOLD_P1[_hq] == 0 and _cum >= 8:
            FOLD_P1[_hq] = _base + _i + 1
        if FOLD_P2[_hq] == 0 and _cum >= 12:
            FOLD_P2[_hq] = _base + _i + 1
        if FOLD_P2B[_hq] == 0 and _cum >= 14:
            FOLD_P2B[_hq] = _base + _i + 1

_CACHE = {}


def _build_bass():
    import concourse.bass as bass
    import concourse.mybir as mybir
    from contextlib import ExitStack

    f32 = mybir.dt.float32
    bf16 = mybir.dt.bfloat16

    nc = bass.Bass()
    kt_in = nc.declare_dram_parameter("kt_in", [HPC, D, S], bf16, isOutput=False)
    qt_in = nc.declare_dram_parameter("qt_in", [HPC, D, S], bf16, isOutput=False)
    va_in = nc.declare_dram_parameter("va_in", [HPC, 128, NT, D], bf16, isOutput=False)
    kq0_in = nc.declare_dram_parameter("kq0_in", [128, 2048], bf16, isOutput=False)
    ctx_out = nc.declare_dram_parameter("ctx_out", [HPC, D, S], bf16, isOutput=True)
    s_out = nc.declare_dram_parameter("s_out", [HPC, NQC, 128, QC], bf16, isOutput=True)

    ctx = ExitStack()
    with ctx:
        kt_sb = ctx.enter_context(nc.sbuf_tensor("kt_sb", [128, HPC, S], bf16))
        qt_sb = ctx.enter_context(nc.sbuf_tensor("qt_sb", [128, HPC, S], bf16))
        # prologue staging: qt0[:, 0:512] ++ kt0[:, 0:1536] (one DRAM slab ->
        # large per-partition descriptors -> fastest first-byte path)
        kq0_sb = ctx.enter_context(nc.sbuf_tensor("kq0_sb", [128, 2048], bf16))
        va_sb = ctx.enter_context(nc.sbuf_tensor("va_sb", [128, HPC, NT, D], bf16))
        pt_sb = [
            ctx.enter_context(nc.sbuf_tensor(f"pt_sb{i}", [128, NT, QC], bf16))
            for i in range(3)
        ]
        s2_sb = ctx.enter_context(nc.sbuf_tensor("s2_sb", [128, 8, QC], bf16))
        u_sb = ctx.enter_context(nc.sbuf_tensor("u_sb", [128, 4, QC], bf16))
        s1_sb = ctx.enter_context(nc.sbuf_tensor("s1_sb", [128, 2, QC], bf16))
        out_sb = ctx.enter_context(nc.sbuf_tensor("out_sb", [128, 2, QC], bf16))
        warm_sb = ctx.enter_context(nc.sbuf_tensor("warm_sb", [128, 2], f32))

        psum_qk = ctx.enter_context(nc.psum_tensor("psum_qk", [128, NQKBUF * QC], f32))
        psum_cx = ctx.enter_context(nc.psum_tensor("psum_cx", [128, 2 * QC], f32))

        load_sems = [ctx.enter_context(nc.semaphore(f"load_sem{i}")) for i in range(10)]
        qk_sem = ctx.enter_context(nc.semaphore("qk_sem"))
        exp_sem = ctx.enter_context(nc.semaphore("exp_sem"))
        sqk_sem = ctx.enter_context(nc.semaphore("sqk_sem"))
        dexp_sem = ctx.enter_context(nc.semaphore("dexp_sem"))
        pv_sem = ctx.enter_context(nc.semaphore("pv_sem"))
        tree_sem = ctx.enter_context(nc.semaphore("tree_sem"))
        cp_sem = ctx.enter_context(nc.semaphore("cp_sem"))
        st_sems = [ctx.enter_context(nc.semaphore(f"st_sem{i}")) for i in range(2)]
        s1st_sems = [ctx.enter_context(nc.semaphore(f"s1st_sem{i}")) for i in range(2)]

        block = ctx.enter_context(nc.Block(no_gpsimd_drain=True))

        @block.sync
        def _(sync):
            # Priority-phased input loads on the HWDGE (sync) path. The SDMA
            # engines round-robin between all queued transfers at packet
            # granularity, so lower-priority loads are gated behind qk_sem
            # progress to give the critical slices exclusive bandwidth.
            # Most urgent first: the staging slab slice QK(0..2) needs, then
            # the rest of the slab (tiles 3:12). Packet-level round-robin
            # favors small in-flight transfers, and the bulky va0/head-1
            # loads are additionally gated on pipeline progress.
            sync.dma_start(out=kq0_sb[:, 0:896], in_=kq0_in[:, 0:896]).then_inc(
                load_sems[0], 16
            )
            sync.dma_start(out=kq0_sb[:, 896:2048], in_=kq0_in[:, 896:2048]).then_inc(
                load_sems[7], 16
            )
            sync.dma_start(out=kt_sb[:, 0, 1536:S], in_=kt_in[0][:, 1536:S]).then_inc(
                load_sems[1], 16
            )
            sync.dma_start(out=qt_sb[:, 0, QC:S], in_=qt_in[0][:, QC:S]).then_inc(
                load_sems[3], 16
            )
            sync.dma_start(out=va_sb[:, 0, 0:4, :], in_=va_in[0][:, 0:4, :]).then_inc(
                load_sems[2], 16
            )
            sync.dma_start(out=va_sb[:, 0, 4:NT, :], in_=va_in[0][:, 4:NT, :]).then_inc(
                load_sems[8], 16
            )
            sync.wait_ge(exp_sem, 10)      # head-1 loads have ~30us of slack
            sync.dma_start(out=kt_sb[:, 1, :], in_=kt_in[1]).then_inc(load_sems[4], 16)
            sync.dma_start(out=qt_sb[:, 1, :], in_=qt_in[1]).then_inc(load_sems[6], 16)
            sync.dma_start(out=va_sb[:, 1, :, :], in_=va_in[1]).then_inc(load_sems[5], 16)
            # Output ctx DMAs from here (sync is idle after the loads);
            # the s1 DMAs stay on gpsimd so the triggers run in parallel.
            for hq in range(NHQ):
                h, qc = divmod(hq, NQC)
                sync.wait_ge(cp_sem, hq + 1)
                sync.dma_start(
                    out=ctx_out[h][:, qc * QC : (qc + 1) * QC],
                    in_=out_sb[:, hq % 2, :],
                ).then_inc(st_sems[hq % 2], 16)
            sync.wait_ge(st_sems[0], 16 * (NHQ // 2))
            sync.wait_ge(st_sems[1], 16 * (NHQ // 2))

        @block.tensor
        def _(tensor):
            def qk_operands(hq, kt):
                h, qc = divmod(hq, NQC)
                if h == 0 and kt < 12:
                    lhs = kq0_sb[:, 512 + kt * 128 : 512 + (kt + 1) * 128]
                else:
                    lhs = kt_sb[:, h, kt * 128 : (kt + 1) * 128]
                if h == 0 and qc == 0:
                    rhs = kq0_sb[:, 0:512]
                else:
                    rhs = qt_sb[:, h, qc * QC : (qc + 1) * QC]
                return lhs, rhs

            def emit_qk(seq):
                hq, kt = divmod(seq, SPC)
                if seq == 0:
                    tensor.wait_ge(load_sems[0], 16)   # kq0 first slice (qt c0 + kt 0:3)
                if seq == 3:
                    tensor.wait_ge(load_sems[7], 16)   # kq0 rest (kt 3:12)
                if seq == 12:
                    tensor.wait_ge(load_sems[1], 16)   # kt0 tiles 12:16
                if seq == SPC:
                    tensor.wait_ge(load_sems[3], 16)   # qt0 rest
                if seq == NQC * SPC:
                    tensor.wait_ge(load_sems[4], 16)   # kt1
                    tensor.wait_ge(load_sems[6], 16)   # qt1
                lhs, rhs = qk_operands(hq, kt)
                b = seq % NQKBUF
                tensor.matmul(
                    psum_qk[:, b * QC : (b + 1) * QC],
                    lhs,
                    rhs,
                    start=True,
                    stop=True,
                ).then_inc(qk_sem, 1)

            def emit_spare_qk(hq, kt):
                # an offloaded tile's QK goes to the idle psum_cx spare bank
                if kt == DVE_KT:
                    if hq >= 1:
                        tensor.wait_ge(cp_sem, hq)       # spare bank drained
                else:
                    tensor.wait_ge(dexp_sem, 2 * hq + 1)  # tile-14 TS read out
                lhs, rhs = qk_operands(hq, kt)
                sp = (hq + 1) % 2
                tensor.matmul(
                    psum_cx[:, sp * QC : (sp + 1) * QC],
                    lhs,
                    rhs,
                    start=True,
                    stop=True,
                ).then_inc(sqk_sem, 1)

            def emit_pv(hq2, kt2):
                h2 = hq2 // NQC
                cb = hq2 % 2
                pb2 = hq2 % 3
                if kt2 == 0:
                    tensor.wait_ge(load_sems[2] if h2 == 0 else load_sems[5], 16)
                    if hq2 >= 2:
                        tensor.wait_ge(cp_sem, hq2 - 1)  # psum_cx buf drained
                    if hq2 >= 1:
                        tensor.wait_ge(dexp_sem, 2 * hq2)  # spare bank read out
                if kt2 == 4 and hq2 == 0:
                    tensor.wait_ge(load_sems[8], 16)     # va0 tiles 4:16
                if kt2 >= DVE_KT:
                    tensor.wait_ge(dexp_sem, 2 * hq2 + (kt2 - DVE_KT) + 1)
                mm = tensor.matmul(
                    psum_cx[:, cb * QC : (cb + 1) * QC],
                    va_sb[:, h2, kt2, :],
                    pt_sb[pb2][:, kt2, :],
                    start=(kt2 == 0),
                    stop=(kt2 == NT - 1),
                )
                if kt2 == NT - 1:
                    mm.then_inc(pv_sem, 1)

            # Warm the PE HAM clock gate (1.2 -> 2.4 GHz needs ~3.4us of
            # sustained activity): dummy matmuls on resident SBUF garbage
            # into psum bank 0, overwritten later by QK g=0 (start=True).
            for _ in range(5):
                tensor.matmul(
                    psum_qk[:, 0:QC],
                    kq0_sb[:, 0:128],
                    kq0_sb[:, 0:QC],
                    start=True,
                    stop=True,
                )
            for seq in range(NQKBUF):
                emit_qk(seq)
            for gi, (s0, n, hq, kt0, o) in enumerate(GRPS):
                tensor.wait_ge(exp_sem, gi + 1)
                for seq in range(s0 + NQKBUF, min(s0 + NQKBUF + n, NHQ * SPC)):
                    emit_qk(seq)
                last_o = 3 if hq == NHQ - 1 else len(_pattern(hq)) - 1
                if o == 2:
                    emit_spare_qk(hq, DVE_KT)
                if o == last_o:
                    emit_spare_qk(hq, DVE_KT + 1)
                if o == 0 and hq >= 1:
                    # previous chunk's offloaded-tile PVs, deferred here so
                    # their dexp waits never block that chunk's QK refills
                    emit_pv(hq - 1, DVE_KT)
                    emit_pv(hq - 1, DVE_KT + 1)
                for j in range(n):
                    emit_pv(hq, kt0 + j)
                if hq == NHQ - 1 and o == len(_pattern(hq)) - 1:
                    # final chunk: its offloaded-tile PVs inline (both TS ran
                    # mid-window) so the stop matmul lands right at exp-end
                    emit_pv(hq, DVE_KT)
                    emit_pv(hq, DVE_KT + 1)

        @block.scalar
        def _(scalar):
            import concourse.mybir as mybir_

            # Warm the Exp activation table immediately: the input is a
            # preamble-initialized const AP, so no waits and no memset chain.
            scalar.activation(
                out=warm_sb[:, 1:2],
                in_=nc.const_aps.tensor(0.0, (128, 1), mybir_.dt.float32),
                func=mybir_.ActivationFunctionType.Exp,
                scale=1.0,
            )
            for gi, (s0, n, hq, kt0, o) in enumerate(GRPS):
                if kt0 == 0 and hq >= 3:
                    # pt buf reused from (hq-3): cp_sem (the ctx copy) implies
                    # both PV and the denominator fold are done
                    scalar.wait_ge(cp_sem, hq - 2)
                scalar.wait_ge(qk_sem, s0 + n)
                b0 = s0 % NQKBUF
                scalar.activation(
                    out=pt_sb[hq % 3][:, kt0 : kt0 + n, :],
                    in_=psum_qk[:, b0 * QC : (b0 + n) * QC],
                    func=mybir_.ActivationFunctionType.Exp,
                    scale=SCALE,
                ).then_inc(exp_sem, 1)
            # The last chunk's ctxT copy runs here (scalar is idle after its
            # final exp; Copy shares the exp table set, so no table reload)
            # in parallel with the final denominator fold on the DVE.
            scalar.wait_ge(pv_sem, NHQ)
            scalar.wait_ge(st_sems[(NHQ - 1) % 2], 16 * ((NHQ - 1) // 2))
            scalar.copy(
                out=out_sb[:, (NHQ - 1) % 2, :],
                in_=psum_cx[:, ((NHQ - 1) % 2) * QC : ((NHQ - 1) % 2 + 1) * QC],
            ).then_inc(cp_sem, 1)

        @block.vector
        def _(vector):
            # Denominator: per chunk, a binary slab fold of the 16 exp planes
            # (mostly FD=1024 tensor_adds), phased in 3 steps so the bulk
            # overlaps the chunk's own exp stream:
            #   P1 (>=8 planes):  a = p0:2+p2:4 -> s2[0:2]; b = p4:6+p6:8 -> s2[2:4]
            #   P2 (>=12 planes): c = p8:10+p10:12 -> s2[4:6]; u0 = a+b -> u[0:2]
            #   P3 (all 16):      d = p12:14+p14:16 -> s2[6:8]; u1 = c+d -> u[2:4];
            #                     w = u0+u1 -> s2[0:2]; s1 = w0+w1
            # then the ctxT PSUM -> SBUF (bf16) staging copy.
            for hq in range(NHQ):
                pb = hq % 3
                sb1 = hq % 2
                cb = hq % 2
                last = hq == NHQ - 1
                pt = pt_sb[pb]
                # this chunk's offloaded k-tiles 14/15: fast-exp on the DVE
                # from the psum_cx spare bank (outside the 6-bank rotation)
                sp = (hq + 1) % 2

                def dve_exp(dk):
                    vector.wait_ge(sqk_sem, 2 * hq + (dk - DVE_KT) + 1)
                    vector.tensor_scalar(
                        out=pt[:, dk : dk + 1, :].bitcast(mybir.dt.int16),
                        in0=psum_cx[:, sp * QC : (sp + 1) * QC],
                        scalar1=A_FEXP,
                        scalar2=B_FEXP,
                        op0=mybir.AluOpType.mult,
                        op1=mybir.AluOpType.add,
                    ).then_inc(dexp_sem, 1)

                dve_exp(DVE_KT)
                with nc.allow_low_precision("bf16 partials; host fp32 final sum"):
                    vector.wait_ge(exp_sem, FOLD_P1[hq])
                    vector.tensor_add(
                        out=s2_sb[:, 0:2, :], in0=pt[:, 0:2, :], in1=pt[:, 2:4, :]
                    )
                    vector.tensor_add(
                        out=s2_sb[:, 2:4, :], in0=pt[:, 4:6, :], in1=pt[:, 6:8, :]
                    )
                    if last:
                        # final chunk: tile-15 fast-exp right after P1 (its
                        # spare QK was emitted early, at ord 3)
                        dve_exp(DVE_KT + 1)
                    vector.wait_ge(exp_sem, FOLD_P2[hq])
                    vector.tensor_add(
                        out=s2_sb[:, 4:6, :], in0=pt[:, 8:10, :], in1=pt[:, 10:12, :]
                    )
                    vector.tensor_add(
                        out=u_sb[:, 0:2, :], in0=s2_sb[:, 0:2, :], in1=s2_sb[:, 2:4, :]
                    )
                    if last:
                        # pre-compute m = p0..11 and t = p14+p15 (DVE planes,
                        # ready mid-window) so only 3 small FD=512 adds remain
                        # after the final exp group: d1 = p12+p13; q = m+d1;
                        # s1 = q+t
                        vector.tensor_add(
                            out=u_sb[:, 3, :], in0=s2_sb[:, 4, :], in1=s2_sb[:, 5, :]
                        )
                        vector.tensor_add(
                            out=s2_sb[:, 6, :], in0=u_sb[:, 0, :], in1=u_sb[:, 1, :]
                        )
                        vector.tensor_add(
                            out=s2_sb[:, 5, :], in0=u_sb[:, 3, :], in1=s2_sb[:, 6, :]
                        )
                    # tile-15 fast-exp here (its spare QK is emitted near the
                    # chunk's last group, after the tile-14 TS freed the bank)
                    if not last:
                        dve_exp(DVE_KT + 1)
                    if last:
                        vector.tensor_add(
                            out=s2_sb[:, 7, :], in0=pt[:, 14, :], in1=pt[:, 15, :]
                        )
                    # final fold phase first: it only needs the chunk's last
                    # exp group (~3us before PV+copy), so the s1 store never
                    # backs up behind the pv-gated ctx copy.
                    vector.wait_ge(exp_sem, GRPS_END[hq])
                    if hq >= 2:
                        vector.wait_ge(s1st_sems[hq % 2], 16 * (hq // 2))
                    if last:
                        vector.tensor_add(
                            out=u_sb[:, 2, :], in0=pt[:, 12, :], in1=pt[:, 13, :]
                        )
                        vector.tensor_add(
                            out=s2_sb[:, 4, :], in0=u_sb[:, 2, :], in1=s2_sb[:, 5, :]
                        )
                        vector.tensor_add(
                            out=s1_sb[:, sb1, :], in0=s2_sb[:, 4, :], in1=s2_sb[:, 7, :]
                        ).then_inc(tree_sem, 1)
                    else:
                        vector.tensor_add(
                            out=s2_sb[:, 6:8, :], in0=pt[:, 12:14, :], in1=pt[:, 14:16, :]
                        )
                        vector.tensor_add(
                            out=u_sb[:, 2:4, :], in0=s2_sb[:, 4:6, :], in1=s2_sb[:, 6:8, :]
                        )
                        vector.tensor_add(
                            out=s2_sb[:, 0:2, :], in0=u_sb[:, 0:2, :], in1=u_sb[:, 2:4, :]
                        )
                        vector.tensor_add(
                            out=s1_sb[:, sb1, :], in0=s2_sb[:, 0, :], in1=s2_sb[:, 1, :]
                        ).then_inc(tree_sem, 1)
                    # ctxT PSUM -> SBUF bf16 staging copy (the last chunk's
                    # copy is done by the scalar engine instead)
                    if not last:
                        vector.wait_ge(pv_sem, hq + 1)
                        if hq >= 2:
                            vector.wait_ge(st_sems[hq % 2], 16 * (hq // 2))
                        vector.tensor_scalar_add(
                            out=out_sb[:, cb, :],
                            in0=psum_cx[:, cb * QC : (cb + 1) * QC],
                            scalar1=0.0,
                        ).then_inc(cp_sem, 1)

        @block.gpsimd
        def _(gpsimd):
            for hq in range(NHQ):
                h, qc = divmod(hq, NQC)
                gpsimd.wait_ge(tree_sem, hq + 1)
                gpsimd.dma_start(
                    out=s_out[h, qc], in_=s1_sb[:, hq % 2, :]
                ).then_inc(s1st_sems[hq % 2], 16)
            gpsimd.wait_ge(s1st_sems[0], 16 * (NHQ // 2))
            gpsimd.wait_ge(s1st_sems[1], 16 * (NHQ // 2))

    return nc


def _perm_blocks(x, idx):
    xb = x.reshape(B, S // BLOCK, BLOCK, H, D)
    return xb[:, idx].reshape(B, S, H, D)


def kernel(**inputs):
    from concourse.bass_utils import run_bass_kernel_spmd

    q = np.asarray(inputs["query"], dtype=np.float32)
    k = np.asarray(inputs["key"], dtype=np.float32)
    v = np.asarray(inputs["value"], dtype=np.float32)
    hp = np.asarray(inputs["head_perm_idx"]).astype(np.int64)
    hd = np.asarray(inputs["head_deperm_idx"]).astype(np.int64)
    rp = np.asarray(inputs["new_row_perm_idx"]).astype(np.int64)
    cp = np.asarray(inputs["new_col_perm_idx"]).astype(np.int64)
    rd = np.asarray(inputs["new_row_deperm_idx"]).astype(np.int64)

    qp = _perm_blocks(q[:, :, hp], rp)[0]  # [S, H, D]
    kp = _perm_blocks(k[:, :, hp], cp)[0]
    vp = _perm_blocks(v[:, :, hp], cp)[0]

    bf = ml_dtypes.bfloat16
    qt = np.ascontiguousarray(qp.transpose(1, 2, 0)).astype(bf)  # [H, D, S]
    kt = np.ascontiguousarray(kp.transpose(1, 2, 0)).astype(bf)  # [H, D, S]
    # va[h, kp, kt, d] = V[h, kt*128 + kp, d]
    va = np.ascontiguousarray(
        vp.transpose(1, 0, 2).reshape(H, NT, 128, D).transpose(0, 2, 1, 3)
    ).astype(bf)

    if "nc" not in _CACHE:
        _CACHE["nc"] = _build_bass()
    nc = _CACHE["nc"]

    core_ids = list(range(NCORES))
    in_maps = [
        {
            "kt_in": np.ascontiguousarray(kt[c * HPC : (c + 1) * HPC]),
            "qt_in": np.ascontiguousarray(qt[c * HPC : (c + 1) * HPC]),
            "va_in": np.ascontiguousarray(va[c * HPC : (c + 1) * HPC]),
            "kq0_in": np.ascontiguousarray(
                np.concatenate(
                    [qt[c * HPC][:, 0:QC], kt[c * HPC][:, 0:1536]], axis=1
                )
            ),
        }
        for c in core_ids
    ]
    res = run_bass_kernel_spmd(nc, in_maps, core_ids)
    _CACHE["last_result"] = res

    ctxT = np.concatenate(
        [np.asarray(res.results[c]["ctx_out"], dtype=np.float32) for c in core_ids],
        axis=0,
    )  # [H, D, S] fp32 (from bf16), unnormalized
    s1 = np.concatenate(
        [np.asarray(res.results[c]["s_out"], dtype=np.float32) for c in core_ids],
        axis=0,
    )  # [H, NQC, 128, QC]
    denom = s1.sum(axis=2).reshape(H, S)  # [H, S]
    ctxT = ctxT / denom[:, None, :]
    ctx = np.ascontiguousarray(ctxT.transpose(2, 0, 1))[None]  # [1, S, H, D]
    ctx = _perm_blocks(ctx, rd)
    out = ctx[:, :, hd]
    return np.ascontiguousarray(out, dtype=np.float32)
